# revision 1
# baseline (speedup 1.0000x reference)
"""Trainium2 Bass kernel for nn_NovaLinkPredictor (hetero GraphSAGE link predictor).

8-core SPMD strategy:
  - Users sharded by range: 8 x 25088 rows (padded 200704). Movies: 8 x 10112 (padded 80896).
  - Edges bucketed by src-range (user side) and globally dst-sorted (movie side), on host.
  - Segment-sums computed on device with one-hot scatter matmuls (S^T @ G) accumulated in
    PSUM per 128-node tile; gathers via dma_gather (int16 idx, 3-range split for movie tables).
  - conv1 movie-side aggregation degenerates: user_x rows are identical (u0), so
    mean = u0 * (cnt_m > 0); cnt_m via a count-only pass + ReduceScatter.
  - Tables exchanged between cores with AllGather / AllToAll collectives (bf16).
  - Final edge dots: labels bucketed by user-range; gather user_o (local) + movie_o (AG'd).

The device program structure (loop bounds) is derived from max-over-core chunk counts so a
single SPMD program serves all 8 cores; per-core data (indices, one-hot keys) comes via inputs.
"""
import sys
sys.path.insert(0, "/opt/trn_rl_repo")
import numpy as np
import ml_dtypes

from concourse import bass, mybir, bacc, tile
from concourse.bass_utils import run_bass_kernel_spmd
from concourse.masks import make_identity

# ---------------- constants ----------------
H = 128
NU = 200000
NM = 80000
FD = 512
W = 8
P = 128

USR = 25088            # user rows per core (196 tiles)
UT = 196
NUP = USR * W          # 200704
MSL = 10112            # movie rows per core (79 tiles)
MT = 79
NMP = MSL * W          # 80896
GMT = NMP // P         # 632 global movie tiles

RNG_STARTS = [0, 27008, 54016]          # movie gather ranges (int16-safe)
RNG_ENDS = [27008, 54016, NMP]
NRNG = 3
GROUP = 8             # chunks per dma_gather (8*128 = 1024 rows; >1024 rows crashes)
SENT = 200.0           # one-hot sentinel (outside 0..127)

bf16 = mybir.dt.bfloat16
f32 = mybir.dt.float32
f32r = mybir.dt.float32r
i16 = mybir.dt.int16
npbf16 = ml_dtypes.bfloat16


# ---------------- host-side preprocessing ----------------

def _wrap16(idx):
    """int16 stream -> [128, n/16] wrapped layout for dma_gather idxs."""
    n = idx.shape[0]
    assert n % 16 == 0
    w = idx.reshape(n // 16, 16).T.astype(np.int16)      # [16, n/16]
    return np.ascontiguousarray(np.tile(w, (8, 1)))      # [128, n/16]


def _chunk_layout(vals, n_chunks, fill):
    """[n_chunks*128] padded stream -> [128, n_chunks] (partition-major)."""
    a = np.full(n_chunks * P, fill, dtype=vals.dtype)
    a[: len(vals)] = vals
    return np.ascontiguousarray(a.reshape(n_chunks, P).T)


def _segment_streams(gidx_list, loc_list, n_cores):
    """Given per-(core) lists of per-segment (gidx, loc) arrays keyed identically,
    pad each segment to the max-over-cores chunk count. Returns per-core
    (gidx_stream, loc_stream[128, NB]) plus per-segment chunk counts."""
    nseg = len(gidx_list[0])
    seg_chunks = []
    for s in range(nseg):
        mx = max(len(gidx_list[c][s]) for c in range(n_cores))
        seg_chunks.append((mx + P - 1) // P)
    nb = sum(seg_chunks)
    g_streams, l_streams = [], []
    for c in range(n_cores):
        g = np.zeros(nb * P, np.int16)
        l = np.full(nb * P, SENT, np.float32)
        pos = 0
        for s in range(nseg):
            n = len(gidx_list[c][s])
            g[pos: pos + n] = gidx_list[c][s]
            l[pos: pos + n] = loc_list[c][s]
            pos += seg_chunks[s] * P
        g_streams.append(g)
        l_streams.append(np.ascontiguousarray(l.reshape(nb, P).T))
    return g_streams, l_streams, seg_chunks


def preprocess(edge_src, edge_dst, lbl_user, lbl_movie):
    """Shard + sort edges/labels; build device index streams and program structure."""
    S = {}
    edge_src = np.asarray(edge_src).astype(np.int64)
    edge_dst = np.asarray(edge_dst).astype(np.int64)
    lbl_user = np.asarray(lbl_user).astype(np.int64)
    lbl_movie = np.asarray(lbl_movie).astype(np.int64)

    u_core = edge_src // USR
    u_loc = edge_src - u_core * USR

    # ---- Pass B streams: per core, segments = (range r, user tile t) in r-major order ----
    B_g, B_l = [], []          # per core: list of per-segment arrays
    for c in range(W):
        m = u_core == c
        src_l = u_loc[m]
        dst = edge_dst[m]
        rng = np.minimum(dst // 27008, 2)
        tilev = src_l // P
        order = np.lexsort((dst, tilev, rng))
        src_l, dst, rng, tilev = src_l[order], dst[order], rng[order], tilev[order]
        segs_g, segs_l = [], []
        for r in range(NRNG):
            for t in range(UT):
                mm = (rng == r) & (tilev == t)
                segs_g.append((dst[mm] - RNG_STARTS[r]).astype(np.int16))
                segs_l.append((src_l[mm] - t * P).astype(np.float32))
        B_g.append(segs_g)
        B_l.append(segs_l)
    Bg_str, Bl_str, B_seg_chunks = _segment_streams(B_g, B_l, W)
    # chunks per (r, t): B_seg_chunks[r*UT + t]
    S["B_chunks"] = np.array(B_seg_chunks).reshape(NRNG, UT)
    S["NB"] = int(S["B_chunks"].sum())

    # ---- Pass A/C streams: per core, segments = global movie tile g (dst-sorted) ----
    C_g, C_l = [], []
    for c in range(W):
        m = u_core == c
        src_l = u_loc[m]
        dst = edge_dst[m]
        order = np.argsort(dst, kind="stable")
        src_l, dst = src_l[order], dst[order]
        gt = dst // P
        segs_g, segs_l = [], []
        for g in range(GMT):
            lo = np.searchsorted(gt, g)
            hi = np.searchsorted(gt, g + 1)
            segs_g.append(src_l[lo:hi].astype(np.int16))
            segs_l.append((dst[lo:hi] - g * P).astype(np.float32))
        C_g.append(segs_g)
        C_l.append(segs_l)
    Cg_str, Cl_str, C_seg_chunks = _segment_streams(C_g, C_l, W)
    S["C_chunks"] = np.array(C_seg_chunks)          # [GMT]
    S["NC"] = int(S["C_chunks"].sum())

    # ---- Pass D streams: labels by user core, segments = movie range ----
    l_core = lbl_user // USR
    D_u, D_m, D_pos = [], [], []
    for c in range(W):
        m = l_core == c
        idxs = np.nonzero(m)[0]
        ul = (lbl_user[m] - c * USR)
        mv = lbl_movie[m]
        rng = np.minimum(mv // 27008, 2)
        order = np.argsort(rng, kind="stable")
        segs_u, segs_m = [], []
        for r in range(NRNG):
            mm = rng[order] == r
            segs_u.append(ul[order][mm].astype(np.int16))
            segs_m.append((mv[order][mm] - RNG_STARTS[r]).astype(np.int16))
        D_u.append(segs_u)
        D_m.append(segs_m)
        D_pos.append(idxs[order])       # original label index per real stream slot
    # pad segments to max-over-cores
    D_seg_chunks = []
    for r in range(NRNG):
        mx = max(len(D_u[c][r]) for c in range(W))
        D_seg_chunks.append((mx + P - 1) // P)
    S["D_chunks"] = np.array(D_seg_chunks)
    S["ND"] = int(S["D_chunks"].sum())
    Du_str, Dm_str, D_real = [], [], []
    for c in range(W):
        du = np.zeros(S["ND"] * P, np.int16)
        dm = np.zeros(S["ND"] * P, np.int16)
        real = np.full(S["ND"] * P, -1, np.int64)
        pos = 0
        k = 0
        for r in range(NRNG):
            n = len(D_u[c][r])
            du[pos: pos + n] = D_u[c][r]
            dm[pos: pos + n] = D_m[c][r]
            real[pos: pos + n] = D_pos[c][k: k + n]
            k += n
            pos += D_seg_chunks[r] * P
        Du_str.append(du)
        Dm_str.append(dm)
        D_real.append(real)

    iota_rep = np.tile(np.arange(P, dtype=np.float32)[None, :], (P, 4))

    per_core = []
    for c in range(W):
        per_core.append({
            "b_loc": Bl_str[c],
            "b_gidx": _wrap16(Bg_str[c]),
            "c_loc": Cl_str[c],
            "c_gidx": _wrap16(Cg_str[c]),
            "d_uidx": _wrap16(Du_str[c]),
            "d_midx": _wrap16(Dm_str[c]),
            "iota": iota_rep,
        })
    return S, per_core, D_real


def _gather_groups(n_chunks_list):
    """Split a list of per-segment chunk counts into dma_gather groups of <=GROUP chunks,
    never crossing the segment-list boundary. Returns list of group sizes (in chunks)."""
    total = sum(n_chunks_list)
    groups = []
    rem = total
    while rem > 0:
        g = min(GROUP, rem)
        groups.append(g)
        rem -= g
    return groups


# ---------------- device program ----------------

def build_program(S):
    import os
    UPTO = int(os.environ.get('KUPTO', '9'))
    nc = bacc.Bacc("TRN2", target_bir_lowering=False, debug=False, num_devices=W)
    NB, NC, ND = S["NB"], S["NC"], S["ND"]
    B_chunks, C_chunks, D_chunks = S["B_chunks"], S["C_chunks"], S["D_chunks"]

    # ---- kernel I/O ----
    featsT = nc.dram_tensor("featsT", [FD, MSL], f32, kind="ExternalInput")
    wm = nc.dram_tensor("wm", [FD, H], f32, kind="ExternalInput")
    u0 = nc.dram_tensor("u0", [H], f32, kind="ExternalInput")
    wnames = ["bm", "wl1_um", "bl1_um", "wr1_um", "wl1_mu", "bl1_mu", "wr1_mu",
              "wl2_um", "bl2_um", "wr2_um", "wl2_mu", "bl2_mu", "wr2_mu"]
    wt = {}
    for n in wnames:
        shape = [H] if n.startswith("b") else [H, H]
        wt[n] = nc.dram_tensor(n, shape, f32, kind="ExternalInput")
    iota_in = nc.dram_tensor("iota", [P, 4 * P], f32, kind="ExternalInput")
    b_loc = nc.dram_tensor("b_loc", [P, NB], f32, kind="ExternalInput")
    b_gidx = nc.dram_tensor("b_gidx", [P, NB * 8], i16, kind="ExternalInput")
    c_loc = nc.dram_tensor("c_loc", [P, NC], f32, kind="ExternalInput")
    c_gidx = nc.dram_tensor("c_gidx", [P, NC * 8], i16, kind="ExternalInput")
    d_uidx = nc.dram_tensor("d_uidx", [P, ND * 8], i16, kind="ExternalInput")
    d_midx = nc.dram_tensor("d_midx", [P, ND * 8], i16, kind="ExternalInput")
    out = nc.dram_tensor("out", [P, ND], f32, kind="ExternalOutput")

    # ---- internal DRAM ----
    cnt_local = nc.dram_tensor("cnt_local", [GMT, P], f32)          # strip rows = tiles
    cnt_rs = nc.dram_tensor("cnt_rs", [MT, P], f32)
    xcat_slice = nc.dram_tensor("xcat_slice", [MSL, 2 * H], bf16)
    xcat_full = nc.dram_tensor("xcat_full", [NMP, 2 * H], bf16, addr_space="Shared")
    mht_stash = nc.dram_tensor("mht_stash", [P, MSL], bf16)
    userh = nc.dram_tensor("userh", [USR, H], bf16)
    usero = nc.dram_tensor("usero", [USR, H], bf16)
    partials = nc.dram_tensor("partials", [NMP, H], bf16)
    parts_recv = nc.dram_tensor("parts_recv", [NMP, H], bf16)
    mo_slice = nc.dram_tensor("mo_slice", [MSL, H], bf16)
    mo_full = nc.dram_tensor("mo_full", [NMP, H], bf16, addr_space="Shared")

    rg = [list(range(W))]

    from contextlib import ExitStack
    with tile.TileContext(nc) as tc, ExitStack() as stack:
        cst = stack.enter_context(tc.tile_pool(name="cst", bufs=1))

        # ---------- constants ----------
        iota_t = cst.tile([P, 4 * P], f32)
        nc.sync.dma_start(out=iota_t[:], in_=iota_in[:])
        ones_bf = cst.tile([P, 1], bf16)
        nc.vector.memset(ones_bf[:], 1.0)
        ident_bf = cst.tile([P, P], bf16)
        make_identity(nc, ident_bf[:])
        ones_row = cst.tile([1, P], f32)
        nc.vector.memset(ones_row[:], 1.0)
        u0_col = cst.tile([P, 1], f32)
        nc.sync.dma_start(out=u0_col[:], in_=u0[:, None])

        wtile = {}
        for n in wnames:
            if n.startswith("b"):
                t = cst.tile([1, P], f32, tag=f"w_{n}")
                nc.sync.dma_start(out=t[:], in_=wt[n][None, :])
            else:
                t = cst.tile([P, P], f32, tag=f"w_{n}")
                nc.sync.dma_start(out=t[:], in_=wt[n][:])
            wtile[n] = t
        # casts
        w_r = {}
        for n in ["wr1_um", "wl1_mu"]:
            t = cst.tile([P, P], f32r, tag=f"wr_{n}")
            nc.vector.tensor_copy(out=t[:], in_=wtile[n][:])
            w_r[n] = t
        w_bf = {}
        for n in ["wr2_mu", "wr2_um", "wl2_um", "wl2_mu"]:
            t = cst.tile([P, P], bf16, tag=f"wbf_{n}")
            nc.vector.tensor_copy(out=t[:], in_=wtile[n][:])
            w_bf[n] = t
        wm_r = []
        for k in range(4):
            t = cst.tile([P, H], f32r, tag=f"wm_{k}")
            nc.sync.dma_start(out=t[:], in_=wm[k * P:(k + 1) * P, :].bitcast(f32r))
            wm_r.append(t)
        bm_col = cst.tile([P, 1], f32)
        nc.sync.dma_start(out=bm_col[:], in_=wt["bm"][:, None])
        bl1um_col = cst.tile([P, 1], f32)
        nc.sync.dma_start(out=bl1um_col[:], in_=wt["bl1_um"][:, None])

        # v_row = u0 @ Wl1_um   [1,128]; r1_row = u0 @ Wr1_mu
        with tc.tile_pool(name="psc", bufs=2, space="PSUM") as psc:
            vp = psc.tile([1, P], f32, space="PSUM", tag="vrow")
            nc.tensor.matmul(out=vp[:], lhsT=u0_col[:], rhs=wtile["wl1_um"][:], start=True, stop=True)
            v_row = cst.tile([1, P], f32)
            nc.vector.tensor_copy(out=v_row[:], in_=vp[:])
            rp = psc.tile([1, P], f32, space="PSUM", tag="vrow")
            nc.tensor.matmul(out=rp[:], lhsT=u0_col[:], rhs=wtile["wr1_mu"][:], start=True, stop=True)
            b1_row = cst.tile([1, P], f32)
            nc.vector.tensor_tensor(out=b1_row[:], in0=rp[:], in1=wtile["bl1_mu"][:], op=mybir.AluOpType.add)

            def bcast_row(row_ap, tag):
                ps = psc.tile([P, P], f32, space="PSUM", tag="bcast")
                nc.tensor.matmul(out=ps[:], lhsT=ones_row[:], rhs=row_ap, start=True, stop=True)
                t = cst.tile([P, P], f32, tag=tag)
                nc.vector.tensor_copy(out=t[:], in_=ps[:])
                return t

            Vcast = bcast_row(v_row[:], "Vcast")
            B1cast = bcast_row(b1_row[:], "B1cast")
            B2cast = bcast_row(wtile["bl2_mu"][:], "B2cast")
            B3cast = bcast_row(wtile["bl2_um"][:], "B3cast")

        # ---------- Pass A: movie in-degree counts ----------
        with tc.tile_pool(name="pa_sb", bufs=1) as pa_sb, \
             tc.tile_pool(name="pa_s4", bufs=4) as pa_s4, \
             tc.tile_pool(name="pa_ps", bufs=4, space="PSUM") as pa_ps, \
             tc.tile_pool(name="pa_st", bufs=2) as pa_st:
            cloc_t = pa_sb.tile([P, NC], f32)
            nc.sync.dma_start(out=cloc_t[:], in_=c_loc[:])
            cntstrip = pa_sb.tile([P, GMT], f32)
            pos = 0
            for g in range(GMT):
                n = int(C_chunks[g])
                if n == 0:
                    nc.vector.memset(cntstrip[:, g:g + 1], 0.0)
                    continue
                ps = pa_ps.tile([P, 8], f32, space="PSUM", tag="cnt")
                for c4 in range((n + 3) // 4):
                    nch = min(4, n - c4 * 4)
                    s4 = pa_s4.tile([P, 4 * P], bf16, tag="s4")
                    nc.vector.tensor_tensor(
                        out=s4[:].rearrange("p (k n) -> p k n", k=4),
                        in0=iota_t[:].rearrange("p (k n) -> p k n", k=4),
                        in1=cloc_t[:, pos + c4 * 4: pos + c4 * 4 + 4][:, :, None].to_broadcast([P, 4, P]),
                        op=mybir.AluOpType.is_equal,
                    )
                    for j in range(nch):
                        ch = c4 * 4 + j
                        nc.tensor.matmul(
                            out=ps[:, 0:1],
                            lhsT=s4[:, j * P:(j + 1) * P],
                            rhs=ones_bf[:],
                            start=(ch == 0), stop=(ch == n - 1),
                        )
                nc.vector.tensor_copy(out=cntstrip[:, g:g + 1], in_=ps[:, 0:1])
                pos += n
            # strip [128, GMT] -> DRAM [GMT, 128] (transposed) then ReduceScatter
            st = pa_st.tile([P, GMT], f32)
            nc.vector.tensor_copy(out=st[:], in_=cntstrip[:])
            nc.sync.dma_start(out=cnt_local[:].transpose([1, 0]), in_=st[:])
        nc.gpsimd.collective_compute(
            "ReduceScatter", mybir.AluOpType.add, replica_groups=rg,
            ins=[cnt_local[:].opt()], outs=[cnt_rs[:].opt()])

        if UPTO >= 2:
            # ---------- Stage 0: movie-side tables ----------
            NCT = (MSL + 511) // 512     # 20 col-tiles (last = 384)
            with tc.tile_pool(name="s0_sb", bufs=1) as s0_sb, \
                 tc.tile_pool(name="s0_mx", bufs=2) as s0_mx, \
                 tc.tile_pool(name="s0_ft", bufs=3) as s0_ft, \
                 tc.tile_pool(name="s0_ps", bufs=1, space="PSUM") as s0_ps, \
                 tc.tile_pool(name="s0_pt", bufs=2, space="PSUM") as s0_pt, \
                 tc.tile_pool(name="s0_stg", bufs=3) as s0_stg:
                p1T = s0_sb.tile([P, MSL], bf16)
                A_fm = s0_sb.tile([P, MSL], bf16)
                mhT = s0_sb.tile([P, MSL], bf16)
                cntcols = s0_sb.tile([P, MT], f32)
                nc.sync.dma_start(out=cntcols[:], in_=cnt_rs[:].transpose([1, 0]))
                indcols = s0_sb.tile([P, MT], f32)
                nc.vector.tensor_scalar(
                    out=indcols[:], in0=cntcols[:], scalar1=0.0, scalar2=None,
                    op0=mybir.AluOpType.is_gt)

                for j in range(NCT):
                    c0 = j * 512
                    cw = min(512, MSL - c0)
                    mxps = s0_ps.tile([P, 512], f32, space="PSUM", tag="mx")
                    for k in range(4):
                        ft = s0_ft.tile([P, 512], f32r, tag="ft")
                        nc.sync.dma_start(out=ft[:, :cw], in_=featsT[k * P:(k + 1) * P, c0:c0 + cw].bitcast(f32r))
                        nc.tensor.matmul(out=mxps[:, :cw], lhsT=wm_r[k][:], rhs=ft[:, :cw],
                                         start=(k == 0), stop=(k == 3))
                    mxt = s0_mx.tile([P, 512], f32r, tag="mxt")
                    nc.vector.tensor_tensor(out=mxt[:, :cw], in0=mxps[:, :cw],
                                            in1=bm_col[:].to_broadcast([P, cw]),
                                            op=mybir.AluOpType.add)
                    p1ps = s0_ps.tile([P, 512], f32, space="PSUM", tag="p1")
                    nc.tensor.matmul(out=p1ps[:, :cw], lhsT=w_r["wl1_mu"][:], rhs=mxt[:, :cw],
                                     start=True, stop=True)
                    nc.vector.tensor_copy(out=p1T[:, c0:c0 + cw], in_=p1ps[:, :cw])
                    aps = s0_ps.tile([P, 512], f32, space="PSUM", tag="A")
                    nc.tensor.matmul(out=aps[:, :cw], lhsT=w_r["wr1_um"][:], rhs=mxt[:, :cw],
                                     start=True, stop=True)
                    nc.vector.tensor_tensor(out=A_fm[:, c0:c0 + cw], in0=aps[:, :cw],
                                            in1=bl1um_col[:].to_broadcast([P, cw]),
                                            op=mybir.AluOpType.add)

                # per 128-tile: movie_h row-major then back to feature-major
                for t in range(MT):
                    c0 = t * P
                    tp = s0_pt.tile([P, P], bf16, space="PSUM", tag="tp")
                    nc.tensor.transpose(out=tp[:], in_=A_fm[:, c0:c0 + P], identity=ident_bf[:])
                    term = s0_stg.tile([P, P], f32, tag="term")
                    nc.vector.tensor_tensor(out=term[:], in0=Vcast[:],
                                            in1=indcols[:, t:t + 1].to_broadcast([P, P]),
                                            op=mybir.AluOpType.mult)
                    mhrow = s0_stg.tile([P, P], bf16, tag="mhrow")
                    nc.vector.tensor_tensor(out=mhrow[:], in0=tp[:], in1=term[:],
                                            op=mybir.AluOpType.add)
                    nc.vector.tensor_scalar_max(out=mhrow[:], in0=mhrow[:], scalar1=0.0)
                    tp2 = s0_pt.tile([P, P], bf16, space="PSUM", tag="tp2")
                    nc.tensor.transpose(out=tp2[:], in_=mhrow[:], identity=ident_bf[:])
                    nc.vector.tensor_copy(out=mhT[:, c0:c0 + P], in_=tp2[:])
                nc.sync.dma_start(out=mht_stash[:], in_=mhT[:])

                # p2T = Wl2_mu.T @ mhT  (bf16)
                p2T = s0_sb.tile([P, MSL], bf16)
                for j in range(NCT):
                    c0 = j * 512
                    cw = min(512, MSL - c0)
                    ps = s0_ps.tile([P, 512], f32, space="PSUM", tag="p2")
                    nc.tensor.matmul(out=ps[:, :cw], lhsT=w_bf["wl2_mu"][:], rhs=mhT[:, c0:c0 + cw],
                                     start=True, stop=True)
                    nc.vector.tensor_copy(out=p2T[:, c0:c0 + cw], in_=ps[:, :cw])

                # transpose to row-major X_cat slice and store
                for t in range(MT):
                    c0 = t * P
                    stg = s0_stg.tile([P, 2 * H], bf16, tag="xrow")
                    tp = s0_pt.tile([P, P], bf16, space="PSUM", tag="tp")
                    nc.tensor.transpose(out=tp[:], in_=p1T[:, c0:c0 + P], identity=ident_bf[:])
                    nc.vector.tensor_copy(out=stg[:, 0:H], in_=tp[:])
                    tp2 = s0_pt.tile([P, P], bf16, space="PSUM", tag="tp2")
                    nc.tensor.transpose(out=tp2[:], in_=p2T[:, c0:c0 + P], identity=ident_bf[:])
                    nc.vector.tensor_copy(out=stg[:, H:2 * H], in_=tp2[:])
                    nc.sync.dma_start(out=xcat_slice[c0:c0 + P, :], in_=stg[:])

            nc.gpsimd.collective_compute(
                "AllGather", mybir.AluOpType.bypass, replica_groups=rg,
                ins=[xcat_slice[:].opt()], outs=[xcat_full[:].opt()])

        if UPTO >= 3:
            # ---------- Pass B: user-side fused aggregation ----------
            ACC = 257  # [p1sum 128 | p2sum 128 | cnt 1]
            with tc.tile_pool(name="pb_sb", bufs=1) as pb_sb, \
                 tc.tile_pool(name="pb_s4", bufs=4) as pb_s4, \
                 tc.tile_pool(name="pb_g", bufs=3) as pb_g, \
                 tc.tile_pool(name="pb_gi", bufs=3) as pb_gi, \
                 tc.tile_pool(name="pb_ps", bufs=2, space="PSUM") as pb_ps, \
                 tc.tile_pool(name="pb_pc", bufs=2, space="PSUM") as pb_pc, \
                 tc.tile_pool(name="pb_pt", bufs=3, space="PSUM") as pb_pt, \
                 tc.tile_pool(name="pb_stg", bufs=4) as pb_stg:
                bloc_t = pb_sb.tile([P, NB], f32)
                nc.sync.dma_start(out=bloc_t[:], in_=b_loc[:])
                accB = pb_sb.tile([P, UT * ACC], bf16)
                nc.vector.memset(accB[:], 0.0)

                pos = 0          # global chunk position (stream)
                for r in range(NRNG):
                    table = xcat_full[RNG_STARTS[r]:RNG_ENDS[r], :]
                    sub_chunks = int(B_chunks[r].sum())
                    # gather groups for this sub-pass
                    gpos = 0
                    gbufs = []
                    while gpos < sub_chunks:
                        gn = min(GROUP, sub_chunks - gpos)
                        gb = pb_g.tile([P, GROUP * 2 * H], bf16, tag="gbuf")
                        gi = pb_gi.tile([P, GROUP * 8], i16, tag="gidx")
                        col0 = (pos + gpos) * 8
                        nc.sync.dma_start(out=gi[:, :gn * 8], in_=b_gidx[:, col0: col0 + gn * 8])
                        nc.gpsimd.dma_gather(
                            out_ap=gb[:, :gn * 2 * H].rearrange("p (c n) -> p c n", c=gn),
                            in_ap=table,
                            idxs_ap=gi[:, :gn * 8],
                            num_idxs=gn * P,
                            num_idxs_reg=gn * P,
                            elem_size=2 * H,
                        )
                        gbufs.append((gpos, gn, gb))
                        gpos += gn

                    def get_slot(sub_pos):
                        for g0, gn, gb in gbufs:
                            if g0 <= sub_pos < g0 + gn:
                                return gb, sub_pos - g0
                        raise AssertionError

                    sub_pos = 0
                    for t in range(UT):
                        n = int(B_chunks[r][t])
                        if n == 0:
                            continue
                        ps = pb_ps.tile([P, 2 * H], f32, space="PSUM", tag="ps")
                        pc = pb_pc.tile([P, 8], f32, space="PSUM", tag="pc")
                        for c4 in range((n + 3) // 4):
                            nch = min(4, n - c4 * 4)
                            s4 = pb_s4.tile([P, 4 * P], bf16, tag="s4")
                            cc = pos + sub_pos + c4 * 4
                            nc.vector.tensor_tensor(
                                out=s4[:].rearrange("p (k n) -> p k n", k=4),
                                in0=iota_t[:].rearrange("p (k n) -> p k n", k=4),
                                in1=bloc_t[:, cc: cc + 4][:, :, None].to_broadcast([P, 4, P]),
                                op=mybir.AluOpType.is_equal,
                            )
                            for j in range(nch):
                                ch = c4 * 4 + j
                                gb, slot = get_slot(sub_pos + ch)
                                nc.tensor.matmul(
                                    out=ps[:],
                                    lhsT=s4[:, j * P:(j + 1) * P],
                                    rhs=gb[:, slot * 2 * H:(slot + 1) * 2 * H],
                                    start=(ch == 0), stop=(ch == n - 1),
                                )
                                nc.tensor.matmul(
                                    out=pc[:, 0:1],
                                    lhsT=s4[:, j * P:(j + 1) * P],
                                    rhs=ones_bf[:],
                                    start=(ch == 0), stop=(ch == n - 1),
                                )
                        a0 = t * ACC
                        nc.vector.tensor_tensor(out=accB[:, a0:a0 + 2 * H], in0=ps[:],
                                                in1=accB[:, a0:a0 + 2 * H], op=mybir.AluOpType.add)
                        nc.vector.tensor_tensor(out=accB[:, a0 + 2 * H:a0 + ACC], in0=pc[:, 0:1],
                                                in1=accB[:, a0 + 2 * H:a0 + ACC], op=mybir.AluOpType.add)
                        sub_pos += n
                    pos += sub_chunks

                # ---- epilogue: user_h / user_o per tile ----
                cntv = pb_stg.tile([P, UT], f32, tag="cntv")
                nc.vector.tensor_copy(
                    out=cntv[:],
                    in_=accB[:].rearrange("p (t a) -> p t a", a=ACC)[:, :, 2 * H:2 * H + 1].squeeze(2))
                nc.vector.tensor_scalar_max(out=cntv[:], in0=cntv[:], scalar1=1.0)
                recipv = pb_stg.tile([P, UT], f32, tag="recipv")
                nc.vector.reciprocal(out=recipv[:], in_=cntv[:])

                for t in range(UT):
                    a0 = t * ACC
                    rc = recipv[:, t:t + 1]
                    uh = pb_stg.tile([P, H], bf16, tag="uh")
                    nc.vector.tensor_tensor(out=uh[:], in0=accB[:, a0:a0 + H],
                                            in1=rc.to_broadcast([P, H]), op=mybir.AluOpType.mult)
                    nc.vector.tensor_tensor(out=uh[:], in0=uh[:], in1=B1cast[:],
                                            op=mybir.AluOpType.add)
                    nc.vector.tensor_scalar_max(out=uh[:], in0=uh[:], scalar1=0.0)
                    tp = pb_pt.tile([P, P], bf16, space="PSUM", tag="ep")
                    nc.tensor.transpose(out=tp[:], in_=uh[:], identity=ident_bf[:])
                    uht = pb_stg.tile([P, P], bf16, tag="uhts")
                    nc.vector.tensor_copy(out=uht[:], in_=tp[:])
                    # Pass C gather table = user_h @ Wl2_um (pre-multiplied; linearity)
                    t2ps = pb_pt.tile([P, P], f32, space="PSUM", tag="ep")
                    nc.tensor.matmul(out=t2ps[:], lhsT=uht[:], rhs=w_bf["wl2_um"][:],
                                     start=True, stop=True)
                    uh2 = pb_stg.tile([P, P], bf16, tag="uh2")
                    nc.vector.tensor_copy(out=uh2[:], in_=t2ps[:])
                    nc.sync.dma_start(out=userh[t * P:(t + 1) * P, :], in_=uh2[:])
                    # user_o = p2sum*recip + B2cast + uh @ Wr2_mu
                    rps = pb_pt.tile([P, P], f32, space="PSUM", tag="ep")
                    nc.tensor.matmul(out=rps[:], lhsT=uht[:], rhs=w_bf["wr2_mu"][:],
                                     start=True, stop=True)
                    uo = pb_stg.tile([P, H], f32, tag="uo")
                    nc.vector.tensor_tensor(out=uo[:], in0=accB[:, a0 + H:a0 + 2 * H],
                                            in1=rc.to_broadcast([P, H]), op=mybir.AluOpType.mult)
                    nc.vector.tensor_tensor(out=uo[:], in0=uo[:], in1=B2cast[:],
                                            op=mybir.AluOpType.add)
                    uo_bf = pb_stg.tile([P, H], bf16, tag="uobf")
                    nc.vector.tensor_tensor(out=uo_bf[:], in0=uo[:], in1=rps[:],
                                            op=mybir.AluOpType.add)
                    nc.sync.dma_start(out=usero[t * P:(t + 1) * P, :], in_=uo_bf[:])

        if UPTO >= 4:
            # ---------- Pass C: movie-side aggregation of user_h ----------
            with tc.tile_pool(name="pc_sb", bufs=1) as pc_sb, \
                 tc.tile_pool(name="pc_s4", bufs=4) as pc_s4, \
                 tc.tile_pool(name="pc_g", bufs=3) as pc_g, \
                 tc.tile_pool(name="pc_gi", bufs=3) as pc_gi, \
                 tc.tile_pool(name="pc_ps", bufs=2, space="PSUM") as pc_ps, \
                 tc.tile_pool(name="pc_stg", bufs=4) as pc_stg:
                cloc_t = pc_sb.tile([P, NC], f32)
                nc.sync.dma_start(out=cloc_t[:], in_=c_loc[:])

                gpos = 0
                gbufs = []
                while gpos < NC:
                    gn = min(GROUP, NC - gpos)
                    gb = pc_g.tile([P, GROUP * H], bf16, tag="gbuf")
                    gi = pc_gi.tile([P, GROUP * 8], i16, tag="gidx")
                    nc.sync.dma_start(out=gi[:, :gn * 8], in_=c_gidx[:, gpos * 8: (gpos + gn) * 8])
                    nc.gpsimd.dma_gather(
                        out_ap=gb[:, :gn * H].rearrange("p (c n) -> p c n", c=gn),
                        in_ap=userh[:],
                        idxs_ap=gi[:, :gn * 8],
                        num_idxs=gn * P,
                        num_idxs_reg=gn * P,
                        elem_size=H,
                    )
                    gbufs.append((gpos, gn, gb))
                    gpos += gn

                def get_slotC(p_):
                    for g0, gn, gb in gbufs:
                        if g0 <= p_ < g0 + gn:
                            return gb, p_ - g0
                    raise AssertionError

                pos = 0
                for g in range(GMT):
                    n = int(C_chunks[g])
                    if n == 0:
                        stg = pc_stg.tile([P, H], bf16, tag="pstg")
                        nc.vector.memset(stg[:], 0.0)
                        nc.sync.dma_start(out=partials[g * P:(g + 1) * P, :], in_=stg[:])
                        continue
                    ps = pc_ps.tile([P, H], f32, space="PSUM", tag="ps")
                    for c4 in range((n + 3) // 4):
                        nch = min(4, n - c4 * 4)
                        s4 = pc_s4.tile([P, 4 * P], bf16, tag="s4")
                        cc = pos + c4 * 4
                        nc.vector.tensor_tensor(
                            out=s4[:].rearrange("p (k n) -> p k n", k=4),
                            in0=iota_t[:].rearrange("p (k n) -> p k n", k=4),
                            in1=cloc_t[:, cc: cc + 4][:, :, None].to_broadcast([P, 4, P]),
                            op=mybir.AluOpType.is_equal,
                        )
                        for j in range(nch):
                            ch = c4 * 4 + j
                            gb, slot = get_slotC(pos + ch)
                            nc.tensor.matmul(
                                out=ps[:],
                                lhsT=s4[:, j * P:(j + 1) * P],
                                rhs=gb[:, slot * H:(slot + 1) * H],
                                start=(ch == 0), stop=(ch == n - 1),
                            )
                    stg = pc_stg.tile([P, H], bf16, tag="pstg")
                    nc.vector.tensor_copy(out=stg[:], in_=ps[:])
                    nc.sync.dma_start(out=partials[g * P:(g + 1) * P, :], in_=stg[:])
                    pos += n

            nc.gpsimd.collective_compute(
                "AllToAll", mybir.AluOpType.bypass, replica_groups=rg,
                ins=[partials[:].opt()], outs=[parts_recv[:].opt()])

        if UPTO >= 5:
            # ---------- movie_o ----------
            with tc.tile_pool(name="mo_sb", bufs=1) as mo_sb, \
                 tc.tile_pool(name="mo_in", bufs=3) as mo_in, \
                 tc.tile_pool(name="mo_ps", bufs=2, space="PSUM") as mo_ps, \
                 tc.tile_pool(name="mo_stg", bufs=4) as mo_stg:
                cntcols = mo_sb.tile([P, MT], f32)
                nc.sync.dma_start(out=cntcols[:], in_=cnt_rs[:].transpose([1, 0]))
                nc.vector.tensor_scalar_max(out=cntcols[:], in0=cntcols[:], scalar1=1.0)
                recipm = mo_sb.tile([P, MT], f32)
                nc.vector.reciprocal(out=recipm[:], in_=cntcols[:])

                rv = parts_recv[:].rearrange("(s m) h -> s m h", s=W)
                for t in range(MT):
                    pin = mo_in.tile([P, W * H], bf16, tag="pin")
                    nc.sync.dma_start(
                        out=pin[:].rearrange("p (s h) -> p s h", s=W),
                        in_=rv[:, t * P:(t + 1) * P, :].transpose([1, 0, 2]))
                    s1 = mo_stg.tile([P, 4 * H], f32, tag="s1")
                    nc.vector.tensor_tensor(out=s1[:], in0=pin[:, 0:4 * H],
                                            in1=pin[:, 4 * H:8 * H], op=mybir.AluOpType.add)
                    s2 = mo_stg.tile([P, 2 * H], f32, tag="s2")
                    nc.vector.tensor_tensor(out=s2[:], in0=s1[:, 0:2 * H],
                                            in1=s1[:, 2 * H:4 * H], op=mybir.AluOpType.add)
                    s3 = mo_stg.tile([P, H], f32, tag="s3")
                    nc.vector.tensor_tensor(out=s3[:], in0=s2[:, 0:H],
                                            in1=s2[:, H:2 * H], op=mybir.AluOpType.add)
                    # root term
                    mh = mo_in.tile([P, P], bf16, tag="mh")
                    nc.sync.dma_start(out=mh[:], in_=mht_stash[:, t * P:(t + 1) * P])
                    rps = mo_ps.tile([P, P], f32, space="PSUM", tag="mroot")
                    nc.tensor.matmul(out=rps[:], lhsT=mh[:], rhs=w_bf["wr2_um"][:],
                                     start=True, stop=True)
                    mo_t = mo_stg.tile([P, H], f32, tag="mo1")
                    nc.vector.tensor_tensor(out=mo_t[:], in0=s3[:],
                                            in1=recipm[:, t:t + 1].to_broadcast([P, H]),
                                            op=mybir.AluOpType.mult)
                    nc.vector.tensor_tensor(out=mo_t[:], in0=mo_t[:], in1=B3cast[:],
                                            op=mybir.AluOpType.add)
                    mo_bf = mo_stg.tile([P, H], bf16, tag="mo2")
                    nc.vector.tensor_tensor(out=mo_bf[:], in0=mo_t[:], in1=rps[:],
                                            op=mybir.AluOpType.add)
                    nc.sync.dma_start(out=mo_slice[t * P:(t + 1) * P, :], in_=mo_bf[:])

            nc.gpsimd.collective_compute(
                "AllGather", mybir.AluOpType.bypass, replica_groups=rg,
                ins=[mo_slice[:].opt()], outs=[mo_full[:].opt()])

        if UPTO >= 6:
            # ---------- Pass D: label dots ----------
            with tc.tile_pool(name="pd_sb", bufs=1) as pd_sb, \
                 tc.tile_pool(name="pd_g", bufs=4) as pd_g, \
                 tc.tile_pool(name="pd_gi", bufs=4) as pd_gi, \
                 tc.tile_pool(name="pd_stg", bufs=4) as pd_stg:
                outstrip = pd_sb.tile([P, ND], f32)
                pos = 0
                for r in range(NRNG):
                    n_r = int(D_chunks[r])
                    table = mo_full[RNG_STARTS[r]:RNG_ENDS[r], :]
                    gpos = 0
                    while gpos < n_r:
                        gn = min(GROUP, n_r - gpos)
                        gu = pd_g.tile([P, GROUP * H], bf16, tag="gu")
                        gm = pd_g.tile([P, GROUP * H], bf16, tag="gm")
                        giu = pd_gi.tile([P, GROUP * 8], i16, tag="giu")
                        gim = pd_gi.tile([P, GROUP * 8], i16, tag="gim")
                        col0 = (pos + gpos) * 8
                        nc.sync.dma_start(out=giu[:, :gn * 8], in_=d_uidx[:, col0: col0 + gn * 8])
                        nc.sync.dma_start(out=gim[:, :gn * 8], in_=d_midx[:, col0: col0 + gn * 8])
                        nc.gpsimd.dma_gather(
                            out_ap=gu[:, :gn * H].rearrange("p (c n) -> p c n", c=gn),
                            in_ap=usero[:], idxs_ap=giu[:, :gn * 8],
                            num_idxs=gn * P, num_idxs_reg=gn * P, elem_size=H)
                        nc.gpsimd.dma_gather(
                            out_ap=gm[:, :gn * H].rearrange("p (c n) -> p c n", c=gn),
                            in_ap=table, idxs_ap=gim[:, :gn * 8],
                            num_idxs=gn * P, num_idxs_reg=gn * P, elem_size=H)
                        for s in range(gn):
                            pr = pd_stg.tile([P, H], f32, tag="pr")
                            nc.vector.tensor_tensor(out=pr[:], in0=gu[:, s * H:(s + 1) * H],
                                                    in1=gm[:, s * H:(s + 1) * H],
                                                    op=mybir.AluOpType.mult)
                            ch = pos + gpos + s
                            nc.vector.tensor_reduce(
                                out=outstrip[:, ch:ch + 1], in_=pr[:],
                                axis=mybir.AxisListType.X, op=mybir.AluOpType.add)
                        gpos += gn
                    pos += n_r
                nc.sync.dma_start(out=out[:], in_=outstrip[:])
        else:
            with tc.tile_pool(name="dummy", bufs=1) as dp:
                z = dp.tile([P, ND], f32)
                nc.vector.memset(z[:], 0.0)
                nc.sync.dma_start(out=out[:], in_=z[:])

    nc.compile()
    return nc


# ---------------- entry point ----------------

_CACHE = {}
TRACE = False
LAST_EXEC_NS = None
LAST_RESULTS = None


def kernel(movie_feats, user_init, edge_src, edge_dst, lbl_user, lbl_movie, n_users,
           Wm, bm,
           Wl1_um, bl1_um, Wr1_um, Wl1_mu, bl1_mu, Wr1_mu,
           Wl2_um, bl2_um, Wr2_um, Wl2_mu, bl2_mu, Wr2_mu):
    movie_feats = np.asarray(movie_feats, dtype=np.float32)
    S, per_core, D_real = preprocess(edge_src, edge_dst, lbl_user, lbl_movie)

    key = (S["NB"], S["NC"], S["ND"],
           S["B_chunks"].tobytes(), S["C_chunks"].tobytes(), S["D_chunks"].tobytes())
    if key in _CACHE:
        nc = _CACHE[key]
    else:
        nc = build_program(S)
        _CACHE[key] = nc

    featsT = np.zeros((FD, NMP), np.float32)
    featsT[:, :NM] = movie_feats.T

    weights = {
        "wm": np.asarray(Wm, np.float32), "u0": np.asarray(user_init, np.float32),
        "bm": np.asarray(bm, np.float32),
        "wl1_um": np.asarray(Wl1_um, np.float32), "bl1_um": np.asarray(bl1_um, np.float32),
        "wr1_um": np.asarray(Wr1_um, np.float32),
        "wl1_mu": np.asarray(Wl1_mu, np.float32), "bl1_mu": np.asarray(bl1_mu, np.float32),
        "wr1_mu": np.asarray(Wr1_mu, np.float32),
        "wl2_um": np.asarray(Wl2_um, np.float32), "bl2_um": np.asarray(bl2_um, np.float32),
        "wr2_um": np.asarray(Wr2_um, np.float32),
        "wl2_mu": np.asarray(Wl2_mu, np.float32), "bl2_mu": np.asarray(bl2_mu, np.float32),
        "wr2_mu": np.asarray(Wr2_mu, np.float32),
    }

    in_maps = []
    for c in range(W):
        m = {"featsT": np.ascontiguousarray(featsT[:, c * MSL:(c + 1) * MSL])}
        m.update(weights)
        pc = per_core[c]
        m.update({
            "iota": pc["iota"],
            "b_loc": pc["b_loc"], "b_gidx": pc["b_gidx"],
            "c_loc": pc["c_loc"], "c_gidx": pc["c_gidx"],
            "d_uidx": pc["d_uidx"], "d_midx": pc["d_midx"],
        })
        in_maps.append(m)

    global LAST_EXEC_NS, LAST_RESULTS
    res = run_bass_kernel_spmd(nc, in_maps, core_ids=list(range(W)), trace=TRACE)
    LAST_EXEC_NS = res.exec_time_ns
    LAST_RESULTS = res

    EL = len(np.asarray(lbl_user))
    out_full = np.zeros(EL, np.float32)
    for c in range(W):
        vals = res.results[c]["out"].T.reshape(-1)       # stream order
        real = D_real[c]
        mask = real >= 0
        out_full[real[mask]] = vals[mask]
    return out_full



# revision 7
# speedup vs baseline: 1.2482x; 1.2482x over previous
"""Trainium2 Bass kernel for nn_NovaLinkPredictor (hetero GraphSAGE link predictor).

8-core SPMD, v2:
  - Stage0: movie_x/movie_h/xcat tables per movie slice (bf16), biases folded into
    the tables (B1=bl1_mu+u0@Wr1_mu into p1, B2=bl2_mu into p2, B3=bl2_um into userh).
  - Pass B (user-side aggregation): edges sharded by src user range, ordered
    (cohort-of-10-user-tiles, movie-range, tile, dst); per-chunk one-hot scatter
    matmuls accumulate straight in PSUM across all 3 movie ranges (no SBUF acc).
    Degrees/recips come from host bincount; zero-degree users get fake edges to a
    patched [B1|B2] table row so mean+bias stays exact.
  - Pass C (movie-side aggregation): edges sharded by dst movie range; gathers the
    AllGather'd userh table (user_h @ Wl2_um + B3); direct local accumulation
    (no AllToAll / partials round-trip).
  - Pass D: labels sharded by user range; user_o rows come from SBUF-resident
    user_o via one-hot expansion matmuls (no user-side gather); movie rows via
    dma_gather of mo_full; dots via scalar_tensor_tensor accumulate.
  - Collectives: 3 AllGathers (xcat 5.2MB/rank, userh 6.4MB/rank, mo 2.6MB/rank).
"""
import sys
sys.path.insert(0, "/opt/trn_rl_repo")
import numpy as np
import ml_dtypes

from concourse import bass, mybir, bacc, tile
from concourse.bass_utils import run_bass_kernel_spmd
from concourse.masks import make_identity

# ---------------- constants ----------------
H = 128
NU = 200000
NM = 80000
FD = 512
W = 8
P = 128

USR = 25088            # users per core (196 tiles)
UT = 196
NUP = USR * W          # 200704
MSL = 10112            # movies per core (79 tiles)
MT = 79
NMP = MSL * W          # 80896

MRS = [0, 27008, 54016]            # movie-table int16 gather ranges
MRE = [27008, 54016, NMP]
NRM = 3
URSZ = 28672
URS = [k * URSZ for k in range(7)]  # user-table int16 gather ranges
NRU = 7
ZROW_M = 80000         # global movie row patched to [B1|B2] (core 7 local 9216)
ZROW_U = 200000        # global user row whose userh row is B3 (pad user)
SENT = 200.0
COH = 6                # user/movie tiles per psum cohort (6 PSUM banks + 2 epilogue)
GROUP = 16             # chunks per dma_gather call

bf16 = mybir.dt.bfloat16
f32 = mybir.dt.float32
i16 = mybir.dt.int16
npbf16 = ml_dtypes.bfloat16
AF = mybir.ActivationFunctionType


# ---------------- host-side preprocessing ----------------

def _wrap16(idx):
    n = idx.shape[0]
    assert n % 16 == 0
    w = idx.reshape(n // 16, 16).T.astype(np.int16)
    return np.ascontiguousarray(np.tile(w, (8, 1)))


def _col_layout(vals, n_chunks, fill, dtype):
    a = np.full(n_chunks * P, fill, dtype=dtype)
    a[: len(vals)] = vals
    return np.ascontiguousarray(a.reshape(n_chunks, P).T)


def _build_pass_streams(core_of, loc_of, gi_of, rng_of, tile_of, n_rng, n_tiles,
                        coh, min1_r0=True):
    """Generic (cohort, range, tile)-ordered stream builder.

    core_of/loc_of/gi_of/rng_of/tile_of: per-edge arrays (already restricted is
    done by caller via boolean masks per core).
    Returns (chunks[r][t] padded-to-max array, per-core (gidx_stream, loc_stream)).
    """
    cnt = np.zeros((W, n_rng, n_tiles), np.int64)
    np.add.at(cnt, (core_of, rng_of, tile_of), 1)
    mx = cnt.max(axis=0)
    chunks = (mx + P - 1) // P
    if min1_r0:
        chunks[0] = np.maximum(chunks[0], 1)
    return chunks, cnt


def preprocess(edge_src, edge_dst, lbl_user, lbl_movie):
    S = {}
    es = np.asarray(edge_src).astype(np.int64)
    ed = np.asarray(edge_dst).astype(np.int64)
    lu = np.asarray(lbl_user).astype(np.int64)
    lm = np.asarray(lbl_movie).astype(np.int64)

    udeg = np.bincount(es, minlength=NUP)
    mdeg = np.bincount(ed, minlength=NMP)
    recip_u = 1.0 / np.maximum(udeg, 1).astype(np.float64)
    recip_m = 1.0 / np.maximum(mdeg, 1).astype(np.float64)
    S["recip_u"] = [
        np.ascontiguousarray(
            recip_u[c * USR:(c + 1) * USR].reshape(UT, P).T.astype(np.float32))
        for c in range(W)]
    S["recip_m"] = [
        np.ascontiguousarray(
            recip_m[c * MSL:(c + 1) * MSL].reshape(MT, P).T.astype(np.float32))
        for c in range(W)]

    # fake edges: zero-degree users -> ZROW_M ([B1|B2] row); zero-deg movies -> ZROW_U
    fu = np.nonzero(udeg[:NU] == 0)[0]
    fm = np.nonzero(mdeg[:NM] == 0)[0]
    es_a = np.concatenate([es, fu, np.full(len(fm), ZROW_U, np.int64)])
    ed_a = np.concatenate([ed, np.full(len(fu), ZROW_M, np.int64), fm])

    # ---- pass B: shard by src core; order (cohort, r, tile, dst) ----
    u_core = es_a // USR
    u_loc = es_a - u_core * USR
    b_tile = u_loc // P
    b_rng = np.minimum(ed_a // 27008, 2)
    chunksB, cntB = _build_pass_streams(u_core, None, None, b_rng, b_tile, NRM, UT, COH)
    S["chunksB"] = chunksB
    # stream order template: (coh, r, t)
    orderB = []
    n_coh_B = (UT + COH - 1) // COH
    for co in range(n_coh_B):
        tiles = list(range(co * COH, min((co + 1) * COH, UT)))
        for r in range(NRM):
            for t in tiles:
                orderB.append((co, r, t, int(chunksB[r][t])))
    S["orderB"] = orderB
    S["NB"] = int(sum(o[3] for o in orderB))

    Bg, Bl = [], []
    for c in range(W):
        m = u_core == c
        key_t = b_tile[m]
        key_r = b_rng[m]
        key_d = ed_a[m]
        key_coh = key_t // COH
        o = np.lexsort((key_d, key_t, key_r, key_coh))
        st = key_t[o]
        sr = key_r[o]
        sd = key_d[o]
        sl = (u_loc[m][o] - st * P)
        # place into padded stream
        g = np.zeros(S["NB"] * P, np.int16)
        l = np.full(S["NB"] * P, SENT, np.float32)
        pos = 0
        ptr = 0
        for (co, r, t, nch) in orderB:
            n = int(cntB[c, r, t])
            g[pos:pos + n] = (sd[ptr:ptr + n] - MRS[r]).astype(np.int16)
            l[pos:pos + n] = sl[ptr:ptr + n]
            ptr += n
            pos += nch * P
        assert ptr == m.sum()
        cntc = cntB[c]
        g = _mark_trailing_pads(g, orderB, lambda seg: (seg[0], seg[1]),
                                lambda seg: int(cntc[seg[1], seg[2]]))
        Bg.append(_wrap16(g))
        nbp = (S["NB"] + 3) // 4 * 4
        lp = np.full(nbp * P, SENT, np.float32)
        lp[:len(l)] = l
        Bl.append(_col_layout(lp, nbp, SENT, np.float32).astype(npbf16))
    S["b_gidx"] = Bg
    S["b_loc"] = Bl

    # ---- pass C: shard by dst core; order (cohort, r7, tile, src) ----
    m_core = ed_a // MSL
    m_loc = ed_a - m_core * MSL
    c_tile = m_loc // P
    c_rng = np.minimum(es_a // URSZ, NRU - 1)
    chunksC, cntC = _build_pass_streams(m_core, None, None, c_rng, c_tile, NRU, MT, COH)
    S["chunksC"] = chunksC
    orderC = []
    n_coh_C = (MT + COH - 1) // COH
    for co in range(n_coh_C):
        tiles = list(range(co * COH, min((co + 1) * COH, MT)))
        for r in range(NRU):
            for t in tiles:
                orderC.append((co, r, t, int(chunksC[r][t])))
    S["orderC"] = orderC
    S["NC"] = int(sum(o[3] for o in orderC))

    Cg, Cl = [], []
    for c in range(W):
        m = m_core == c
        key_t = c_tile[m]
        key_r = c_rng[m]
        key_s = es_a[m]
        key_coh = key_t // COH
        o = np.lexsort((key_s, key_t, key_r, key_coh))
        st = key_t[o]
        sr = key_r[o]
        ss = key_s[o]
        sl = (m_loc[m][o] - st * P)
        g = np.zeros(S["NC"] * P, np.int16)
        l = np.full(S["NC"] * P, SENT, np.float32)
        pos = 0
        ptr = 0
        for (co, r, t, nch) in orderC:
            n = int(cntC[c, r, t])
            g[pos:pos + n] = (ss[ptr:ptr + n] - URS[r]).astype(np.int16)
            l[pos:pos + n] = sl[ptr:ptr + n]
            ptr += n
            pos += nch * P
        assert ptr == m.sum()
        cntc = cntC[c]
        g = _mark_trailing_pads(g, orderC, lambda seg: (seg[0], seg[1]),
                                lambda seg: int(cntc[seg[1], seg[2]]))
        Cg.append(_wrap16(g))
        ncp = (S["NC"] + 3) // 4 * 4
        lp = np.full(ncp * P, SENT, np.float32)
        lp[:len(l)] = l
        Cl.append(_col_layout(lp, ncp, SENT, np.float32).astype(npbf16))
    S["c_gidx"] = Cg
    S["c_loc"] = Cl

    # ---- pass D: labels by user core; order (r3, tile) ----
    l_core = lu // USR
    l_uloc = lu - l_core * USR
    d_tile = l_uloc // P
    d_rng = np.minimum(lm // 27008, 2)
    cntD = np.zeros((W, NRM, UT), np.int64)
    np.add.at(cntD, (l_core, d_rng, d_tile), 1)
    mxD = cntD.max(axis=0)
    chunksD = (mxD + P - 1) // P
    S["chunksD"] = chunksD
    orderD = []
    for r in range(NRM):
        for t in range(UT):
            orderD.append((r, t, int(chunksD[r][t])))
    S["orderD"] = orderD
    S["ND"] = int(sum(o[2] for o in orderD))

    Dg, Du, Dreal = [], [], []
    for c in range(W):
        m = l_core == c
        idxs = np.nonzero(m)[0]
        kt = d_tile[m]
        kr = d_rng[m]
        o = np.lexsort((kt, kr))
        st = kt[o]
        sm = lm[m][o]
        sr = kr[o]
        sl = (l_uloc[m][o] - st * P)
        g = np.zeros(S["ND"] * P, np.int16)
        ul = np.full(S["ND"] * P, SENT, np.float32)
        real = np.full(S["ND"] * P, -1, np.int64)
        pos = 0
        ptr = 0
        for (r, t, nch) in orderD:
            n = int(cntD[c, r, t])
            g[pos:pos + n] = (sm[ptr:ptr + n] - MRS[r]).astype(np.int16)
            ul[pos:pos + n] = sl[ptr:ptr + n]
            real[pos:pos + n] = idxs[o][ptr:ptr + n]
            ptr += n
            pos += nch * P
        assert ptr == m.sum()
        cntc = cntD[c]
        g = _mark_trailing_pads(g, orderD, lambda seg: seg[0],
                                lambda seg: int(cntc[seg[0], seg[1]]))
        Dg.append(_wrap16(g))
        Du.append(ul.astype(npbf16)[None, :])      # [1, ND*P] row layout
        Dreal.append(real)
    S["d_mgidx"] = Dg
    S["d_uloc"] = Du
    return S, Dreal


# ---------------- device program ----------------

def _gather_groups(order, key_fn):
    """Windows of <=GROUP chunks within contiguous key_fn spans.
    Returns list of (key, chunk0, gn)."""
    groups = []
    for key, c0, nch in _gather_spans(order, key_fn):
        gpos = 0
        while gpos < nch:
            gn = min(GROUP, nch - gpos)
            groups.append((key, c0 + gpos, gn))
            gpos += gn
    return groups


def _mark_trailing_pads(g, order, key_fn, seg_counts_for_core):
    """Set gather idx to -1 for pad slots with no real edge after them inside
    their gather group. g is the flat idx stream [NCH*128]."""
    seg_real = []
    pos = 0
    for seg in order:
        nch = seg[-1]
        n = seg_counts_for_core(seg)
        seg_real.append((pos, pos + n))
        pos += nch * P
    # real mask per slot
    real = np.zeros(len(g), bool)
    for a, b in seg_real:
        real[a:b] = True
    for key, c0, gn in _gather_groups(order, key_fn):
        a, b = c0 * P, (c0 + gn) * P
        w = real[a:b]
        nz = np.nonzero(w)[0]
        if len(nz) == 0:
            g[a] = 0          # keep one valid idx so the call isn't all-negative
            g[a + 1:b] = -1
            continue
        tail = a + int(nz[-1]) + 1
        g[tail:b] = -1        # everything after the last real slot is a pad
    return g


def _gather_spans(order, key_fn):
    """Group stream segments into gather spans keyed by key_fn((co,r,t)).
    Returns list of (key, chunk0, nchunks)."""
    spans = []
    pos = 0
    for seg in order:
        nch = seg[-1]
        k = key_fn(seg)
        if spans and spans[-1][0] == k:
            spans[-1] = (k, spans[-1][1], spans[-1][2] + nch)
        else:
            spans.append((k, pos, nch))
        pos += nch
    return spans


def build_program(S):
    nc = bacc.Bacc("TRN2", target_bir_lowering=False, debug=False, num_devices=W)
    NB, NC, ND = S["NB"], S["NC"], S["ND"]
    orderB, orderC, orderD = S["orderB"], S["orderC"], S["orderD"]

    # ---- kernel I/O ----
    featsT = nc.dram_tensor("featsT", [FD, MSL], bf16, kind="ExternalInput")
    wm4 = nc.dram_tensor("wm4", [FD, H], bf16, kind="ExternalInput")
    wnamesHH = ["wl1mu", "wr1um", "wl2mu", "wl2um", "wr2mu", "wr2um"]
    wt = {n: nc.dram_tensor(n, [H, H], bf16, kind="ExternalInput") for n in wnamesHH}
    bnames = ["bm_col", "b1_col", "bl1um_col", "b2_col"]
    bt = {n: nc.dram_tensor(n, [H], f32, kind="ExternalInput") for n in bnames}
    b3_row = nc.dram_tensor("b3_row", [1, H], bf16, kind="ExternalInput")
    recip_u = nc.dram_tensor("recip_u", [P, UT], f32, kind="ExternalInput")
    recip_m = nc.dram_tensor("recip_m", [P, MT], f32, kind="ExternalInput")
    zmask = nc.dram_tensor("zmask", [1, 1], f32, kind="ExternalInput")
    zfix = nc.dram_tensor("zfix", [1, 2 * H], bf16, kind="ExternalInput")
    iota_in = nc.dram_tensor("iota", [P, 4 * P], bf16, kind="ExternalInput")
    piota_in = nc.dram_tensor("piota", [P, P], f32, kind="ExternalInput")
    NBp = (NB + 3) // 4 * 4
    b_loc = nc.dram_tensor("b_loc", [P, NBp], bf16, kind="ExternalInput")
    b_gidx = nc.dram_tensor("b_gidx", [P, NB * 8], i16, kind="ExternalInput")
    NCp = (NC + 3) // 4 * 4
    c_loc = nc.dram_tensor("c_loc", [P, NCp], bf16, kind="ExternalInput")
    c_gidx = nc.dram_tensor("c_gidx", [P, NC * 8], i16, kind="ExternalInput")
    d_uloc = nc.dram_tensor("d_uloc", [1, ND * P], bf16, kind="ExternalInput")
    d_mgidx = nc.dram_tensor("d_mgidx", [P, ND * 8], i16, kind="ExternalInput")
    out = nc.dram_tensor("out", [P, ND], f32, kind="ExternalOutput")

    # ---- internal DRAM ----
    xcat_slice = nc.dram_tensor("xcat_slice", [MSL, 2 * H], bf16)
    xcat_full = nc.dram_tensor("xcat_full", [NMP, 2 * H], bf16, addr_space="Shared")
    userh_sl = nc.dram_tensor("userh_sl", [USR, H], bf16)
    userh_full = nc.dram_tensor("userh_full", [NUP, H], bf16, addr_space="Shared")
    mo_sl = nc.dram_tensor("mo_sl", [MSL, H], bf16)
    mo_full = nc.dram_tensor("mo_full", [NMP, H], bf16, addr_space="Shared")

    rg = [list(range(W))]

    from contextlib import ExitStack
    with tile.TileContext(nc) as tc, ExitStack() as stack:
        cst = stack.enter_context(tc.tile_pool(name="cst", bufs=1))

        # ---------- constants ----------
        iota_t = cst.tile([P, 4 * P], bf16)
        nc.sync.dma_start(out=iota_t[:], in_=iota_in[:])
        piota_t = cst.tile([P, P], f32)
        nc.sync.dma_start(out=piota_t[:], in_=piota_in[:])
        ident_bf = cst.tile([P, P], bf16)
        make_identity(nc, ident_bf[:])
        ones1_bf = cst.tile([1, P], bf16)
        nc.vector.memset(ones1_bf[:], 1.0)
        ru_t = cst.tile([P, UT], f32)
        nc.sync.dma_start(out=ru_t[:], in_=recip_u[:])
        rm_t = cst.tile([P, MT], f32)
        nc.sync.dma_start(out=rm_t[:], in_=recip_m[:])
        zmask_t = cst.tile([1, 1], f32)
        nc.sync.dma_start(out=zmask_t[:], in_=zmask[:])
        zfix_t = cst.tile([1, 2 * H], bf16)
        nc.sync.dma_start(out=zfix_t[:], in_=zfix[:])
        b3_t = cst.tile([1, H], bf16)
        nc.sync.dma_start(out=b3_t[:], in_=b3_row[:])

        wtile = {}
        for n in wnamesHH:
            t = cst.tile([P, P], bf16, tag=f"w_{n}")
            nc.sync.dma_start(out=t[:], in_=wt[n][:])
            wtile[n] = t
        wm_t = []
        for k in range(4):
            t = cst.tile([P, H], bf16, tag=f"wm_{k}")
            nc.sync.dma_start(out=t[:], in_=wm4[k * P:(k + 1) * P, :])
            wm_t.append(t)
        bcol = {}
        for n in bnames:
            t = cst.tile([P, 1], f32, tag=f"b_{n}")
            nc.sync.dma_start(out=t[:], in_=bt[n][:, None])
            bcol[n] = t

        # mht resident [P, MSL] bf16 (~19.8KB/partition)
        mht = cst.tile([P, MSL], bf16)
        # uo resident [P, UT*H] bf16 (~49KB/partition)
        uo_sb = cst.tile([P, UT * H], bf16)

        # ---------- stage0: movie tables ----------
        NCT = (MSL + 511) // 512
        with nc.named_scope("stage0"), \
             tc.tile_pool(name="s0_ft", bufs=3) as s0_ft, \
             tc.tile_pool(name="s0_sb", bufs=2) as s0_sb, \
             tc.tile_pool(name="s0_ps", bufs=1, space="PSUM") as s0_ps, \
             tc.tile_pool(name="s0_pt", bufs=2, space="PSUM") as s0_pt, \
             tc.tile_pool(name="s0_stg", bufs=3) as s0_stg:
            for j in range(NCT):
                c0 = j * 512
                cw = min(512, MSL - c0)
                mxps = s0_ps.tile([P, 512], f32, space="PSUM", tag="mx")
                for k in range(4):
                    ft = s0_ft.tile([P, 512], bf16, tag="ft")
                    nc.sync.dma_start(out=ft[:, :cw], in_=featsT[k * P:(k + 1) * P, c0:c0 + cw])
                    nc.tensor.matmul(out=mxps[:, :cw], lhsT=wm_t[k][:], rhs=ft[:, :cw],
                                     start=(k == 0), stop=(k == 3))
                mxt = s0_sb.tile([P, 512], bf16, tag="mxt")
                nc.scalar.activation(out=mxt[:, :cw], in_=mxps[:, :cw], func=AF.Identity,
                                     bias=bcol["bm_col"][:])
                p1ps = s0_ps.tile([P, 512], f32, space="PSUM", tag="p1")
                nc.tensor.matmul(out=p1ps[:, :cw], lhsT=wtile["wl1mu"][:], rhs=mxt[:, :cw],
                                 start=True, stop=True)
                p1T = s0_sb.tile([P, 512], bf16, tag="p1T")
                nc.scalar.activation(out=p1T[:, :cw], in_=p1ps[:, :cw], func=AF.Identity,
                                     bias=bcol["b1_col"][:])
                aps = s0_ps.tile([P, 512], f32, space="PSUM", tag="A")
                nc.tensor.matmul(out=aps[:, :cw], lhsT=wtile["wr1um"][:], rhs=mxt[:, :cw],
                                 start=True, stop=True)
                nc.scalar.activation(out=mht[:, c0:c0 + cw], in_=aps[:, :cw], func=AF.Relu,
                                     bias=bcol["bl1um_col"][:])
                p2ps = s0_ps.tile([P, 512], f32, space="PSUM", tag="p2")
                nc.tensor.matmul(out=p2ps[:, :cw], lhsT=wtile["wl2mu"][:], rhs=mht[:, c0:c0 + cw],
                                 start=True, stop=True)
                p2T = s0_sb.tile([P, 512], bf16, tag="p2T")
                nc.scalar.activation(out=p2T[:, :cw], in_=p2ps[:, :cw], func=AF.Identity,
                                     bias=bcol["b2_col"][:])
                for tt in range(cw // 128):
                    gt = j * 4 + tt
                    xrow = s0_stg.tile([P, 2 * H], bf16, tag="xrow")
                    tpa = s0_pt.tile([P, P], bf16, space="PSUM", tag="tpa")
                    nc.tensor.transpose(out=tpa[:], in_=p1T[:, tt * P:(tt + 1) * P],
                                        identity=ident_bf[:])
                    nc.vector.tensor_copy(out=xrow[:, 0:H], in_=tpa[:])
                    tpb = s0_pt.tile([P, P], bf16, space="PSUM", tag="tpb")
                    nc.tensor.transpose(out=tpb[:], in_=p2T[:, tt * P:(tt + 1) * P],
                                        identity=ident_bf[:])
                    nc.scalar.copy(out=xrow[:, H:2 * H], in_=tpb[:])
                    if gt == ZROW_M % MSL // P and True:
                        # patch local row (ZROW_M%MSL)%P of this tile (slot 0 of tile 72)
                        nc.vector.scalar_tensor_tensor(
                            out=xrow[0:1, :], in0=xrow[0:1, :], scalar=zmask_t[0:1, 0:1],
                            in1=zfix_t[0:1, :], op0=mybir.AluOpType.mult,
                            op1=mybir.AluOpType.add)
                    nc.sync.dma_start(out=xcat_slice[gt * P:(gt + 1) * P, :], in_=xrow[:])

        nc.gpsimd.collective_compute(
            "AllGather", mybir.AluOpType.bypass, replica_groups=rg,
            ins=[xcat_slice[:].opt()], outs=[xcat_full[:].opt()])

        # ---------- generic aggregation pass ----------
        def agg_pass(name, order, loc_dram, gidx_dram, n_stream, table_full, tstart,
                     tend, width, n_rng, n_tiles, epilogue, psum_tags):
            """order: list of (coh, r, t, nch). width: rhs free size (256 or 128)."""
            groupsAll = _gather_groups(order, key_fn=lambda seg: (seg[0], seg[1]))
            # chunk -> (group first chunk, tile index of buffer)
            with tc.tile_pool(name=f"{name}_sb", bufs=1) as sbp, \
                 tc.tile_pool(name=f"{name}_s4", bufs=4) as s4p, \
                 tc.tile_pool(name=f"{name}_g", bufs=3) as gp_, \
                 tc.tile_pool(name=f"{name}_gi", bufs=3) as gip, \
                 tc.tile_pool(name=f"{name}_acc", bufs=1, space="PSUM") as accp, \
                 tc.tile_pool(name=f"{name}_eps", bufs=1, space="PSUM") as epsp, \
                 tc.tile_pool(name=f"{name}_est", bufs=3) as estp:
                nsp = (n_stream + 3) // 4 * 4
                loc_t = sbp.tile([P, nsp], bf16)
                nc.sync.dma_start(out=loc_t[:], in_=loc_dram[:])

                # gather buffers, keyed by chunk
                gbufs = {}
                for _i in range(3):
                    zb = gp_.tile([P, GROUP * width], bf16, tag="gb", name="gb")
                    nc.vector.memset(zb[:], 0.0)

                def ensure_gathers(key):
                    for (k_, c0, gn) in groupsAll:
                        if k_ != key:
                            continue
                        r = k_[1]
                        gb = gp_.tile([P, GROUP * width], bf16, tag="gb", name="gb")
                        gi = gip.tile([P, GROUP * 8], i16, tag="gi", name="gi")
                        col0 = c0 * 8
                        nc.sync.dma_start(out=gi[:, :gn * 8],
                                          in_=gidx_dram[:, col0: col0 + gn * 8])
                        nc.gpsimd.dma_gather(
                            out_ap=gb[:, :gn * width].rearrange("p (c n) -> p c n", c=gn),
                            in_ap=table_full[tstart[r]:tend[r], :],
                            idxs_ap=gi[:, :gn * 8],
                            num_idxs=gn * P, num_idxs_reg=gn * P, elem_size=width)
                        for k in range(gn):
                            gbufs[c0 + k] = (gb, k)

                # s4 one-hot cache, keyed by 4-chunk group id
                s4cache = {}

                def get_s4(chunk):
                    g4 = chunk // 4
                    if g4 not in s4cache:
                        s4 = s4p.tile([P, 4 * P], bf16, tag="s4")
                        cc = g4 * 4
                        nc.vector.tensor_tensor(
                            out=s4[:].rearrange("p (k n) -> p k n", k=4),
                            in0=iota_t[:].rearrange("p (k n) -> p k n", k=4),
                            in1=loc_t[:, cc: cc + 4][:, :, None].to_broadcast([P, 4, P]),
                            op=mybir.AluOpType.is_equal)
                        s4cache[g4] = s4
                    return s4cache[g4]

                # per-cohort processing
                n_coh = (n_tiles + COH - 1) // COH
                # segment bookkeeping: positions in stream
                seg_pos = {}
                pos = 0
                for (co, r, t, nch) in order:
                    seg_pos[(r, t)] = (pos, nch)
                    pos += nch

                issued_spans = set()

                pos = 0
                oi = 0
                for co in range(n_coh):
                    tiles = list(range(co * COH, min((co + 1) * COH, n_tiles)))
                    accs = {t: accp.tile([P, width], f32, space="PSUM", name=f"acc{t}",
                                         tag=f"acc{t % COH}") for t in tiles}
                    # last (r, chunk) per tile for stop flag
                    last_chunk = {}
                    first_chunk = {}
                    for r in range(n_rng):
                        for t in tiles:
                            p0, nch = seg_pos[(r, t)]
                            if nch == 0:
                                continue
                            if t not in first_chunk:
                                first_chunk[t] = p0
                            last_chunk[t] = p0 + nch - 1
                    for r in range(n_rng):
                        if (co, r) not in issued_spans:
                            ensure_gathers((co, r))
                            issued_spans.add((co, r))
                        for t in tiles:
                            p0, nch = seg_pos[(r, t)]
                            for i in range(nch):
                                ch = p0 + i
                                gb, slot = gbufs[ch]
                                s4 = get_s4(ch)
                                nc.tensor.matmul(
                                    out=accs[t][:],
                                    lhsT=s4[:, (ch % 4) * P:(ch % 4 + 1) * P],
                                    rhs=gb[:, slot * width:(slot + 1) * width],
                                    start=(ch == first_chunk[t]),
                                    stop=(ch == last_chunk[t]))
                    for t in tiles:
                        epilogue(t, accs[t], epsp, estp)
                        # release gather bufs for this cohort implicitly by pool reuse
                    # drop references so pools can recycle
                    for k in [k for k, v in list(gbufs.items())]:
                        pass

            return

        # ---------- pass B ----------
        def epilogue_B(t, acc, epsp, estp):
            rc = ru_t[:, t:t + 1]
            uh = estp.tile([P, H], bf16, tag="uh")
            nc.scalar.activation(out=uh[:], in_=acc[:, 0:H], func=AF.Relu, scale=rc)
            tp = epsp.tile([P, P], bf16, space="PSUM", tag="tp")
            nc.tensor.transpose(out=tp[:], in_=uh[:], identity=ident_bf[:])
            uht = estp.tile([P, P], bf16, tag="uht")
            nc.vector.tensor_copy(out=uht[:], in_=tp[:])
            psh = epsp.tile([P, H], f32, space="PSUM", tag="ps2")
            nc.tensor.matmul(out=psh[:], lhsT=uht[:], rhs=wtile["wl2um"][:],
                             start=True, stop=False)
            nc.tensor.matmul(out=psh[:], lhsT=ones1_bf[:], rhs=b3_t[:],
                             start=False, stop=True)
            uhsb = estp.tile([P, H], bf16, tag="uhsb")
            nc.vector.tensor_copy(out=uhsb[:], in_=psh[:])
            nc.sync.dma_start(out=userh_sl[t * P:(t + 1) * P, :], in_=uhsb[:])
            psr = epsp.tile([P, H], f32, space="PSUM", tag="ps2")
            nc.tensor.matmul(out=psr[:], lhsT=uht[:], rhs=wtile["wr2mu"][:],
                             start=True, stop=True)
            uo1 = estp.tile([P, H], bf16, tag="uo1")
            nc.scalar.activation(out=uo1[:], in_=acc[:, H:2 * H], func=AF.Copy, scale=rc)
            nc.vector.tensor_tensor(out=uo_sb[:, t * H:(t + 1) * H], in0=uo1[:],
                                    in1=psr[:], op=mybir.AluOpType.add)

        with nc.named_scope("passB"):
            agg_pass("pb", orderB, b_loc, b_gidx, NB, xcat_full, MRS, MRE,
                     2 * H, NRM, UT, epilogue_B, None)

        nc.gpsimd.collective_compute(
            "AllGather", mybir.AluOpType.bypass, replica_groups=rg,
            ins=[userh_sl[:].opt()], outs=[userh_full[:].opt()])

        # ---------- pass C ----------
        def epilogue_C(t, acc, epsp, estp):
            rc = rm_t[:, t:t + 1]
            psr = epsp.tile([P, H], f32, space="PSUM", tag="psr")
            nc.tensor.matmul(out=psr[:], lhsT=mht[:, t * P:(t + 1) * P],
                             rhs=wtile["wr2um"][:], start=True, stop=True)
            mo1 = estp.tile([P, H], bf16, tag="mo1")
            nc.scalar.activation(out=mo1[:], in_=acc[:], func=AF.Copy, scale=rc)
            mo2 = estp.tile([P, H], bf16, tag="mo2")
            nc.vector.tensor_tensor(out=mo2[:], in0=mo1[:], in1=psr[:],
                                    op=mybir.AluOpType.add)
            nc.sync.dma_start(out=mo_sl[t * P:(t + 1) * P, :], in_=mo2[:])

        with nc.named_scope("passC"):
            agg_pass("pc", orderC, c_loc, c_gidx, NC, userh_full,
                     [URS[r] for r in range(NRU)],
                     [URS[r] + URSZ for r in range(NRU)],
                     H, NRU, MT, epilogue_C, None)

        nc.gpsimd.collective_compute(
            "AllGather", mybir.AluOpType.bypass, replica_groups=rg,
            ins=[mo_sl[:].opt()], outs=[mo_full[:].opt()])

        # ---------- pass D ----------
        with nc.named_scope("passD"), \
             tc.tile_pool(name="pd_sb", bufs=1) as pdsb, \
             tc.tile_pool(name="pd_ul", bufs=2) as pdul, \
             tc.tile_pool(name="pd_g", bufs=3) as pdg, \
             tc.tile_pool(name="pd_gi", bufs=3) as pdgi, \
             tc.tile_pool(name="pd_ps", bufs=3, space="PSUM") as pdps, \
             tc.tile_pool(name="pd_st", bufs=4) as pdst:
            outstrip = pdsb.tile([P, ND], f32)

            # gather spans: contiguous (r) ranges
            groupsD = _gather_groups(orderD, key_fn=lambda seg: seg[0])
            gbufsD = {}
            for _i in range(3):
                zb = pdg.tile([P, GROUP * H], bf16, tag="gb", name="gb")
                nc.vector.memset(zb[:], 0.0)

            def ensure_gathers_D(r):
                for (k_, c0, gn) in groupsD:
                    if k_ != r:
                        continue
                    gb = pdg.tile([P, GROUP * H], bf16, tag="gb", name="gb")
                    gi = pdgi.tile([P, GROUP * 8], i16, tag="gi", name="gi")
                    col0 = c0 * 8
                    nc.sync.dma_start(out=gi[:, :gn * 8],
                                      in_=d_mgidx[:, col0: col0 + gn * 8])
                    nc.gpsimd.dma_gather(
                        out_ap=gb[:, :gn * H].rearrange("p (c n) -> p c n", c=gn),
                        in_ap=mo_full[MRS[r]:MRE[r], :],
                        idxs_ap=gi[:, :gn * 8],
                        num_idxs=gn * P, num_idxs_reg=gn * P, elem_size=H)
                    for k in range(gn):
                        gbufsD[c0 + k] = (gb, k)

            # uloc row pieces of 32 chunks
            ULW = 32

            ul_cache = {}

            def get_ul(chunk):
                blk = chunk // ULW
                if blk not in ul_cache:
                    w = min(ULW * P, ND * P - blk * ULW * P)
                    ul = pdul.tile([1, ULW * P], bf16, tag="ul")
                    nc.sync.dma_start(out=ul[0:1, :w],
                                      in_=d_uloc[0:1, blk * ULW * P: blk * ULW * P + w])
                    ul_cache[blk] = ul
                return ul_cache[blk]

            pos = 0
            issuedD = set()
            for (r, t, nch) in orderD:
                if r not in issuedD:
                    ensure_gathers_D(r)
                    issuedD.add(r)
                for i in range(nch):
                    ch = pos + i
                    gbm, slot = gbufsD[ch]
                    ul = get_ul(ch)
                    off = (ch % ULW) * P
                    psb = pdps.tile([P, P], f32, space="PSUM", tag="bc")
                    nc.tensor.matmul(out=psb[:], lhsT=ones1_bf[:],
                                     rhs=ul[0:1, off:off + P], start=True, stop=True)
                    E = pdst.tile([P, P], bf16, tag="E")
                    nc.vector.tensor_tensor(out=E[:], in0=piota_t[:], in1=psb[:],
                                            op=mybir.AluOpType.is_equal)
                    gu = pdps.tile([P, H], f32, space="PSUM", tag="gu")
                    nc.tensor.matmul(out=gu[:], lhsT=E[:],
                                     rhs=uo_sb[:, t * H:(t + 1) * H], start=True, stop=True)
                    scr = pdst.tile([P, H], bf16, tag="scr")
                    nc.vector.scalar_tensor_tensor(
                        out=scr[:], in0=gu[:], scalar=1.0,
                        in1=gbm[:, slot * H:(slot + 1) * H],
                        op0=mybir.AluOpType.mult, op1=mybir.AluOpType.mult,
                        accum_out=outstrip[:, ch:ch + 1])
                pos += nch
            nc.sync.dma_start(out=out[:], in_=outstrip[:])

    nc.compile()
    return nc


# ---------------- entry point ----------------

_CACHE = {}
TRACE = False
LAST_EXEC_NS = None
LAST_RESULTS = None


def kernel(movie_feats, user_init, edge_src, edge_dst, lbl_user, lbl_movie, n_users,
           Wm, bm,
           Wl1_um, bl1_um, Wr1_um, Wl1_mu, bl1_mu, Wr1_mu,
           Wl2_um, bl2_um, Wr2_um, Wl2_mu, bl2_mu, Wr2_mu):
    movie_feats = np.asarray(movie_feats, dtype=np.float32)
    u0 = np.asarray(user_init, np.float32)
    S, Dreal = preprocess(edge_src, edge_dst, lbl_user, lbl_movie)

    key = (S["NB"], S["NC"], S["ND"],
           S["chunksB"].tobytes(), S["chunksC"].tobytes(), S["chunksD"].tobytes())
    if key in _CACHE:
        nc = _CACHE[key]
    else:
        nc = build_program(S)
        _CACHE[key] = nc

    featsT = np.zeros((FD, NMP), npbf16)
    featsT[:, :NM] = movie_feats.T.astype(npbf16)

    # folded biases (host): B1 = bl1_mu + u0 @ Wr1_mu ; B2 = bl2_mu ; B3 = bl2_um
    B1 = (np.asarray(bl1_mu, np.float64) +
          u0.astype(np.float64) @ np.asarray(Wr1_mu, np.float64)).astype(np.float32)
    B2 = np.asarray(bl2_mu, np.float32)
    B3 = np.asarray(bl2_um, np.float32)

    iota_rep = np.tile(np.arange(P, dtype=np.float32)[None, :], (P, 4)).astype(npbf16)
    piota = np.tile(np.arange(P, dtype=np.float32)[:, None], (1, P))

    weights = {
        "wm4": np.asarray(Wm, np.float32).astype(npbf16),
        "wl1mu": np.asarray(Wl1_mu, np.float32).astype(npbf16),
        "wr1um": np.asarray(Wr1_um, np.float32).astype(npbf16),
        "wl2mu": np.asarray(Wl2_mu, np.float32).astype(npbf16),
        "wl2um": np.asarray(Wl2_um, np.float32).astype(npbf16),
        "wr2mu": np.asarray(Wr2_mu, np.float32).astype(npbf16),
        "wr2um": np.asarray(Wr2_um, np.float32).astype(npbf16),
        "bm_col": np.asarray(bm, np.float32),
        "b1_col": B1,
        "bl1um_col": np.asarray(bl1_um, np.float32),
        "b2_col": B2,
        "b3_row": B3.astype(npbf16)[None, :],
        "iota": iota_rep,
        "piota": piota.astype(np.float32),
    }

    zfix_row = np.concatenate([B1, B2]).astype(npbf16)[None, :]

    in_maps = []
    for c in range(W):
        m = {"featsT": np.ascontiguousarray(featsT[:, c * MSL:(c + 1) * MSL])}
        m.update(weights)
        m.update({
            "recip_u": S["recip_u"][c], "recip_m": S["recip_m"][c],
            "zmask": np.array([[0.0 if c == ZROW_M // MSL else 1.0]], np.float32),
            "zfix": zfix_row if c == ZROW_M // MSL else np.zeros((1, 2 * H), npbf16),
            "b_loc": S["b_loc"][c], "b_gidx": S["b_gidx"][c],
            "c_loc": S["c_loc"][c], "c_gidx": S["c_gidx"][c],
            "d_uloc": S["d_uloc"][c], "d_mgidx": S["d_mgidx"][c],
        })
        in_maps.append(m)

    global LAST_EXEC_NS, LAST_RESULTS
    res = run_bass_kernel_spmd(nc, in_maps, core_ids=list(range(W)), trace=TRACE)
    LAST_EXEC_NS = res.exec_time_ns
    LAST_RESULTS = res

    EL = len(np.asarray(lbl_user))
    out_full = np.zeros(EL, np.float32)
    for c in range(W):
        vals = res.results[c]["out"].T.reshape(-1)
        real = Dreal[c]
        mask = real >= 0
        out_full[real[mask]] = vals[mask]
    return out_full


# revision 12
# speedup vs baseline: 1.2508x; 1.0020x over previous
"""Trainium2 Bass kernel for nn_NovaLinkPredictor (hetero GraphSAGE link predictor).

8-core SPMD, v2:
  - Stage0: movie_x/movie_h/xcat tables per movie slice (bf16), biases folded into
    the tables (B1=bl1_mu+u0@Wr1_mu into p1, B2=bl2_mu into p2, B3=bl2_um into userh).
  - Pass B (user-side aggregation): edges sharded by src user range, ordered
    (cohort-of-10-user-tiles, movie-range, tile, dst); per-chunk one-hot scatter
    matmuls accumulate straight in PSUM across all 3 movie ranges (no SBUF acc).
    Degrees/recips come from host bincount; zero-degree users get fake edges to a
    patched [B1|B2] table row so mean+bias stays exact.
  - Pass C (movie-side aggregation): edges sharded by dst movie range; gathers the
    AllGather'd userh table (user_h @ Wl2_um + B3); direct local accumulation
    (no AllToAll / partials round-trip).
  - Pass D: labels sharded by user range; user_o rows come from SBUF-resident
    user_o via one-hot expansion matmuls (no user-side gather); movie rows via
    dma_gather of mo_full; dots via scalar_tensor_tensor accumulate.
  - Collectives: 3 AllGathers (xcat 5.2MB/rank, userh 6.4MB/rank, mo 2.6MB/rank).
"""
import sys
sys.path.insert(0, "/opt/trn_rl_repo")
import numpy as np
import ml_dtypes

from concourse import bass, mybir, bacc, tile
from concourse.bass_utils import run_bass_kernel_spmd
from concourse.masks import make_identity

# ---------------- constants ----------------
H = 128
NU = 200000
NM = 80000
FD = 512
W = 8
P = 128

USR = 25088            # users per core (196 tiles)
UT = 196
NUP = USR * W          # 200704
MSL = 10112            # movies per core (79 tiles)
MT = 79
NMP = MSL * W          # 80896

MRS = [0, 27008, 54016]            # movie-table int16 gather ranges
MRE = [27008, 54016, NMP]
NRM = 3
URSZ = 28672
URS = [k * URSZ for k in range(7)]  # user-table int16 gather ranges
NRU = 7
ZROW_M = 80000         # global movie row patched to [B1|B2] (core 7 local 9216)
ZROW_U = 200000        # global user row whose userh row is B3 (pad user)
SENT = 200.0
COH = 6                # user/movie tiles per psum cohort (6 PSUM banks + 2 epilogue)
GROUP = 8              # chunks per dma_gather call

bf16 = mybir.dt.bfloat16
f32 = mybir.dt.float32
i16 = mybir.dt.int16
npbf16 = ml_dtypes.bfloat16
AF = mybir.ActivationFunctionType


# ---------------- host-side preprocessing ----------------

def _wrap16(idx):
    n = idx.shape[0]
    assert n % 16 == 0
    w = idx.reshape(n // 16, 16).T.astype(np.int16)
    return np.ascontiguousarray(np.tile(w, (8, 1)))


def _col_layout(vals, n_chunks, fill, dtype):
    a = np.full(n_chunks * P, fill, dtype=dtype)
    a[: len(vals)] = vals
    return np.ascontiguousarray(a.reshape(n_chunks, P).T)


def _build_pass_streams(core_of, loc_of, gi_of, rng_of, tile_of, n_rng, n_tiles,
                        coh, min1_r0=True):
    """Generic (cohort, range, tile)-ordered stream builder.

    core_of/loc_of/gi_of/rng_of/tile_of: per-edge arrays (already restricted is
    done by caller via boolean masks per core).
    Returns (chunks[r][t] padded-to-max array, per-core (gidx_stream, loc_stream)).
    """
    cnt = np.zeros((W, n_rng, n_tiles), np.int64)
    np.add.at(cnt, (core_of, rng_of, tile_of), 1)
    mx = cnt.max(axis=0)
    chunks = (mx + P - 1) // P
    if min1_r0:
        chunks[0] = np.maximum(chunks[0], 1)
    return chunks, cnt


def preprocess(edge_src, edge_dst, lbl_user, lbl_movie):
    S = {}
    es = np.asarray(edge_src).astype(np.int64)
    ed = np.asarray(edge_dst).astype(np.int64)
    lu = np.asarray(lbl_user).astype(np.int64)
    lm = np.asarray(lbl_movie).astype(np.int64)

    udeg = np.bincount(es, minlength=NUP)
    mdeg = np.bincount(ed, minlength=NMP)
    recip_u = 1.0 / np.maximum(udeg, 1).astype(np.float64)
    recip_m = 1.0 / np.maximum(mdeg, 1).astype(np.float64)
    S["recip_u"] = [
        np.ascontiguousarray(
            recip_u[c * USR:(c + 1) * USR].reshape(UT, P).T.astype(np.float32))
        for c in range(W)]
    S["recip_m"] = [
        np.ascontiguousarray(
            recip_m[c * MSL:(c + 1) * MSL].reshape(MT, P).T.astype(np.float32))
        for c in range(W)]

    # fake edges: zero-degree users -> ZROW_M ([B1|B2] row); zero-deg movies -> ZROW_U
    fu = np.nonzero(udeg[:NU] == 0)[0]
    fm = np.nonzero(mdeg[:NM] == 0)[0]
    es_a = np.concatenate([es, fu, np.full(len(fm), ZROW_U, np.int64)])
    ed_a = np.concatenate([ed, np.full(len(fu), ZROW_M, np.int64), fm])

    # ---- pass B: shard by src core; order (cohort, r, tile, dst) ----
    u_core = es_a // USR
    u_loc = es_a - u_core * USR
    b_tile = u_loc // P
    b_rng = np.minimum(ed_a // 27008, 2)
    chunksB, cntB = _build_pass_streams(u_core, None, None, b_rng, b_tile, NRM, UT, COH)
    S["chunksB"] = chunksB
    # stream order template: (coh, r, t)
    orderB = []
    n_coh_B = (UT + COH - 1) // COH
    for co in range(n_coh_B):
        tiles = list(range(co * COH, min((co + 1) * COH, UT)))
        for r in range(NRM):
            for t in tiles:
                orderB.append((co, r, t, int(chunksB[r][t])))
    S["orderB"] = orderB
    S["NB"] = int(sum(o[3] for o in orderB))

    Bg, Bl = [], []
    for c in range(W):
        m = u_core == c
        key_t = b_tile[m]
        key_r = b_rng[m]
        key_d = ed_a[m]
        key_coh = key_t // COH
        o = np.lexsort((key_d, key_t, key_r, key_coh))
        st = key_t[o]
        sr = key_r[o]
        sd = key_d[o]
        sl = (u_loc[m][o] - st * P)
        # place into padded stream
        g = np.zeros(S["NB"] * P, np.int16)
        l = np.full(S["NB"] * P, SENT, np.float32)
        pos = 0
        ptr = 0
        for (co, r, t, nch) in orderB:
            n = int(cntB[c, r, t])
            g[pos:pos + n] = (sd[ptr:ptr + n] - MRS[r]).astype(np.int16)
            l[pos:pos + n] = sl[ptr:ptr + n]
            ptr += n
            pos += nch * P
        assert ptr == m.sum()
        cntc = cntB[c]
        g = _mark_trailing_pads(g, orderB, lambda seg: (seg[0], seg[1]),
                                lambda seg: int(cntc[seg[1], seg[2]]))
        Bg.append(_wrap16(g))
        nbp = (S["NB"] + 3) // 4 * 4
        lp = np.full(nbp * P, SENT, np.float32)
        lp[:len(l)] = l
        Bl.append(_col_layout(lp, nbp, SENT, np.float32).astype(npbf16))
    S["b_gidx"] = Bg
    S["b_loc"] = Bl

    # ---- pass C: shard by dst core; order (cohort, r7, tile, src) ----
    m_core = ed_a // MSL
    m_loc = ed_a - m_core * MSL
    c_tile = m_loc // P
    c_rng = np.minimum(es_a // URSZ, NRU - 1)
    chunksC, cntC = _build_pass_streams(m_core, None, None, c_rng, c_tile, NRU, MT, COH)
    S["chunksC"] = chunksC
    orderC = []
    n_coh_C = (MT + COH - 1) // COH
    for co in range(n_coh_C):
        tiles = list(range(co * COH, min((co + 1) * COH, MT)))
        for r in range(NRU):
            for t in tiles:
                orderC.append((co, r, t, int(chunksC[r][t])))
    S["orderC"] = orderC
    S["NC"] = int(sum(o[3] for o in orderC))

    Cg, Cl = [], []
    for c in range(W):
        m = m_core == c
        key_t = c_tile[m]
        key_r = c_rng[m]
        key_s = es_a[m]
        key_coh = key_t // COH
        o = np.lexsort((key_s, key_t, key_r, key_coh))
        st = key_t[o]
        sr = key_r[o]
        ss = key_s[o]
        sl = (m_loc[m][o] - st * P)
        g = np.zeros(S["NC"] * P, np.int16)
        l = np.full(S["NC"] * P, SENT, np.float32)
        pos = 0
        ptr = 0
        for (co, r, t, nch) in orderC:
            n = int(cntC[c, r, t])
            g[pos:pos + n] = (ss[ptr:ptr + n] - URS[r]).astype(np.int16)
            l[pos:pos + n] = sl[ptr:ptr + n]
            ptr += n
            pos += nch * P
        assert ptr == m.sum()
        cntc = cntC[c]
        g = _mark_trailing_pads(g, orderC, lambda seg: (seg[0], seg[1]),
                                lambda seg: int(cntc[seg[1], seg[2]]))
        Cg.append(_wrap16(g))
        ncp = (S["NC"] + 3) // 4 * 4
        lp = np.full(ncp * P, SENT, np.float32)
        lp[:len(l)] = l
        Cl.append(_col_layout(lp, ncp, SENT, np.float32).astype(npbf16))
    S["c_gidx"] = Cg
    S["c_loc"] = Cl

    # ---- pass D: labels by user core; order (r3, tile) ----
    l_core = lu // USR
    l_uloc = lu - l_core * USR
    d_tile = l_uloc // P
    d_rng = np.minimum(lm // 27008, 2)
    cntD = np.zeros((W, NRM, UT), np.int64)
    np.add.at(cntD, (l_core, d_rng, d_tile), 1)
    mxD = cntD.max(axis=0)
    chunksD = (mxD + P - 1) // P
    S["chunksD"] = chunksD
    orderD = []
    for r in range(NRM):
        for t in range(UT):
            orderD.append((r, t, int(chunksD[r][t])))
    S["orderD"] = orderD
    S["ND"] = int(sum(o[2] for o in orderD))

    Dg, Du, Dreal = [], [], []
    for c in range(W):
        m = l_core == c
        idxs = np.nonzero(m)[0]
        kt = d_tile[m]
        kr = d_rng[m]
        o = np.lexsort((kt, kr))
        st = kt[o]
        sm = lm[m][o]
        sr = kr[o]
        sl = (l_uloc[m][o] - st * P)
        g = np.zeros(S["ND"] * P, np.int16)
        ul = np.full(S["ND"] * P, SENT, np.float32)
        real = np.full(S["ND"] * P, -1, np.int64)
        pos = 0
        ptr = 0
        for (r, t, nch) in orderD:
            n = int(cntD[c, r, t])
            g[pos:pos + n] = (sm[ptr:ptr + n] - MRS[r]).astype(np.int16)
            ul[pos:pos + n] = sl[ptr:ptr + n]
            real[pos:pos + n] = idxs[o][ptr:ptr + n]
            ptr += n
            pos += nch * P
        assert ptr == m.sum()
        cntc = cntD[c]
        g = _mark_trailing_pads(g, orderD, lambda seg: seg[0],
                                lambda seg: int(cntc[seg[0], seg[1]]))
        Dg.append(_wrap16(g))
        Du.append(ul.astype(npbf16)[None, :])      # [1, ND*P] row layout
        Dreal.append(real)
    S["d_mgidx"] = Dg
    S["d_uloc"] = Du
    return S, Dreal


# ---------------- device program ----------------

def _gather_groups(order, key_fn):
    """Windows of <=GROUP chunks within contiguous key_fn spans.
    Returns list of (key, chunk0, gn)."""
    groups = []
    for key, c0, nch in _gather_spans(order, key_fn):
        gpos = 0
        while gpos < nch:
            gn = min(GROUP, nch - gpos)
            groups.append((key, c0 + gpos, gn))
            gpos += gn
    return groups


def _mark_trailing_pads(g, order, key_fn, seg_counts_for_core):
    return g

    """Set gather idx to -1 for pad slots with no real edge after them inside
    their gather group. g is the flat idx stream [NCH*128]."""
    seg_real = []
    pos = 0
    for seg in order:
        nch = seg[-1]
        n = seg_counts_for_core(seg)
        seg_real.append((pos, pos + n))
        pos += nch * P
    # real mask per slot
    real = np.zeros(len(g), bool)
    for a, b in seg_real:
        real[a:b] = True
    for key, c0, gn in _gather_groups(order, key_fn):
        a, b = c0 * P, (c0 + gn) * P
        w = real[a:b]
        nz = np.nonzero(w)[0]
        if len(nz) == 0:
            g[a] = 0          # keep one valid idx so the call isn't all-negative
            g[a + 1:b] = -1
            continue
        tail = a + int(nz[-1]) + 1
        g[tail:b] = -1        # everything after the last real slot is a pad
    return g


def _gather_spans(order, key_fn):
    """Group stream segments into gather spans keyed by key_fn((co,r,t)).
    Returns list of (key, chunk0, nchunks)."""
    spans = []
    pos = 0
    for seg in order:
        nch = seg[-1]
        k = key_fn(seg)
        if spans and spans[-1][0] == k:
            spans[-1] = (k, spans[-1][1], spans[-1][2] + nch)
        else:
            spans.append((k, pos, nch))
        pos += nch
    return spans


def build_program(S):
    nc = bacc.Bacc("TRN2", target_bir_lowering=False, debug=False, num_devices=W)
    NB, NC, ND = S["NB"], S["NC"], S["ND"]
    orderB, orderC, orderD = S["orderB"], S["orderC"], S["orderD"]

    # ---- kernel I/O ----
    featsT = nc.dram_tensor("featsT", [FD, MSL], bf16, kind="ExternalInput")
    wm4 = nc.dram_tensor("wm4", [FD, H], bf16, kind="ExternalInput")
    wnamesHH = ["wl1mu", "wr1um", "wl2mu", "wl2um", "wr2mu", "wr2um"]
    wt = {n: nc.dram_tensor(n, [H, H], bf16, kind="ExternalInput") for n in wnamesHH}
    bnames = ["bm_col", "b1_col", "bl1um_col", "b2_col"]
    bt = {n: nc.dram_tensor(n, [H], f32, kind="ExternalInput") for n in bnames}
    b3_row = nc.dram_tensor("b3_row", [1, H], bf16, kind="ExternalInput")
    recip_u = nc.dram_tensor("recip_u", [P, UT], f32, kind="ExternalInput")
    recip_m = nc.dram_tensor("recip_m", [P, MT], f32, kind="ExternalInput")
    zmask = nc.dram_tensor("zmask", [1, 1], f32, kind="ExternalInput")
    zfix = nc.dram_tensor("zfix", [1, 2 * H], bf16, kind="ExternalInput")
    iota_in = nc.dram_tensor("iota", [P, 4 * P], bf16, kind="ExternalInput")
    piota_in = nc.dram_tensor("piota", [P, P], f32, kind="ExternalInput")
    NBp = (NB + 3) // 4 * 4
    b_loc = nc.dram_tensor("b_loc", [P, NBp], bf16, kind="ExternalInput")
    b_gidx = nc.dram_tensor("b_gidx", [P, NB * 8], i16, kind="ExternalInput")
    NCp = (NC + 3) // 4 * 4
    c_loc = nc.dram_tensor("c_loc", [P, NCp], bf16, kind="ExternalInput")
    c_gidx = nc.dram_tensor("c_gidx", [P, NC * 8], i16, kind="ExternalInput")
    d_uloc = nc.dram_tensor("d_uloc", [1, ND * P], bf16, kind="ExternalInput")
    d_mgidx = nc.dram_tensor("d_mgidx", [P, ND * 8], i16, kind="ExternalInput")
    out = nc.dram_tensor("out", [P, ND], f32, kind="ExternalOutput")

    # ---- internal DRAM ----
    xcat_slice = nc.dram_tensor("xcat_slice", [MSL, 2 * H], bf16)
    xcat_full = nc.dram_tensor("xcat_full", [NMP, 2 * H], bf16, addr_space="Shared")
    userh_sl = nc.dram_tensor("userh_sl", [USR, H], bf16)
    userh_full = nc.dram_tensor("userh_full", [NUP, H], bf16, addr_space="Shared")
    mo_sl = nc.dram_tensor("mo_sl", [MSL, H], bf16)
    mo_full = nc.dram_tensor("mo_full", [NMP, H], bf16, addr_space="Shared")

    rg = [list(range(W))]

    from contextlib import ExitStack
    with tile.TileContext(nc) as tc, ExitStack() as stack:
        cst = stack.enter_context(tc.tile_pool(name="cst", bufs=1))

        # ---------- constants ----------
        iota_t = cst.tile([P, 4 * P], bf16)
        nc.sync.dma_start(out=iota_t[:], in_=iota_in[:])
        piota_t = cst.tile([P, P], f32)
        nc.sync.dma_start(out=piota_t[:], in_=piota_in[:])
        ident_bf = cst.tile([P, P], bf16)
        make_identity(nc, ident_bf[:])
        ones1_bf = cst.tile([1, P], bf16)
        nc.vector.memset(ones1_bf[:], 1.0)
        ru_t = cst.tile([P, UT], f32)
        nc.sync.dma_start(out=ru_t[:], in_=recip_u[:])
        rm_t = cst.tile([P, MT], f32)
        nc.sync.dma_start(out=rm_t[:], in_=recip_m[:])
        zmask_t = cst.tile([1, 1], f32)
        nc.sync.dma_start(out=zmask_t[:], in_=zmask[:])
        zfix_t = cst.tile([1, 2 * H], bf16)
        nc.sync.dma_start(out=zfix_t[:], in_=zfix[:])
        b3_t = cst.tile([1, H], bf16)
        nc.sync.dma_start(out=b3_t[:], in_=b3_row[:])

        wtile = {}
        for n in wnamesHH:
            t = cst.tile([P, P], bf16, tag=f"w_{n}")
            nc.sync.dma_start(out=t[:], in_=wt[n][:])
            wtile[n] = t
        wm_t = []
        for k in range(4):
            t = cst.tile([P, H], bf16, tag=f"wm_{k}")
            nc.sync.dma_start(out=t[:], in_=wm4[k * P:(k + 1) * P, :])
            wm_t.append(t)
        bcol = {}
        for n in bnames:
            t = cst.tile([P, 1], f32, tag=f"b_{n}")
            nc.sync.dma_start(out=t[:], in_=bt[n][:, None])
            bcol[n] = t

        # mht resident [P, MSL] bf16 (~19.8KB/partition)
        mht = cst.tile([P, MSL], bf16)
        # uo resident [P, UT*H] bf16 (~49KB/partition)
        uo_sb = cst.tile([P, UT * H], bf16)

        # ---------- stage0: movie tables ----------
        NCT = (MSL + 511) // 512
        with nc.named_scope("stage0"), \
             tc.tile_pool(name="s0_ft", bufs=3) as s0_ft, \
             tc.tile_pool(name="s0_sb", bufs=2) as s0_sb, \
             tc.tile_pool(name="s0_ps", bufs=1, space="PSUM") as s0_ps, \
             tc.tile_pool(name="s0_pt", bufs=2, space="PSUM") as s0_pt, \
             tc.tile_pool(name="s0_stg", bufs=3) as s0_stg:
            for j in range(NCT):
                c0 = j * 512
                cw = min(512, MSL - c0)
                mxps = s0_ps.tile([P, 512], f32, space="PSUM", tag="mx")
                for k in range(4):
                    ft = s0_ft.tile([P, 512], bf16, tag="ft")
                    nc.sync.dma_start(out=ft[:, :cw], in_=featsT[k * P:(k + 1) * P, c0:c0 + cw])
                    nc.tensor.matmul(out=mxps[:, :cw], lhsT=wm_t[k][:], rhs=ft[:, :cw],
                                     start=(k == 0), stop=(k == 3))
                mxt = s0_sb.tile([P, 512], bf16, tag="mxt")
                nc.scalar.activation(out=mxt[:, :cw], in_=mxps[:, :cw], func=AF.Identity,
                                     bias=bcol["bm_col"][:])
                p1ps = s0_ps.tile([P, 512], f32, space="PSUM", tag="p1")
                nc.tensor.matmul(out=p1ps[:, :cw], lhsT=wtile["wl1mu"][:], rhs=mxt[:, :cw],
                                 start=True, stop=True)
                p1T = s0_sb.tile([P, 512], bf16, tag="p1T")
                nc.scalar.activation(out=p1T[:, :cw], in_=p1ps[:, :cw], func=AF.Identity,
                                     bias=bcol["b1_col"][:])
                aps = s0_ps.tile([P, 512], f32, space="PSUM", tag="A")
                nc.tensor.matmul(out=aps[:, :cw], lhsT=wtile["wr1um"][:], rhs=mxt[:, :cw],
                                 start=True, stop=True)
                nc.scalar.activation(out=mht[:, c0:c0 + cw], in_=aps[:, :cw], func=AF.Relu,
                                     bias=bcol["bl1um_col"][:])
                p2ps = s0_ps.tile([P, 512], f32, space="PSUM", tag="p2")
                nc.tensor.matmul(out=p2ps[:, :cw], lhsT=wtile["wl2mu"][:], rhs=mht[:, c0:c0 + cw],
                                 start=True, stop=True)
                p2T = s0_sb.tile([P, 512], bf16, tag="p2T")
                nc.scalar.activation(out=p2T[:, :cw], in_=p2ps[:, :cw], func=AF.Identity,
                                     bias=bcol["b2_col"][:])
                for tt in range(cw // 128):
                    gt = j * 4 + tt
                    xrow = s0_stg.tile([P, 2 * H], bf16, tag="xrow")
                    tpa = s0_pt.tile([P, P], bf16, space="PSUM", tag="tpa")
                    nc.tensor.transpose(out=tpa[:], in_=p1T[:, tt * P:(tt + 1) * P],
                                        identity=ident_bf[:])
                    nc.vector.tensor_copy(out=xrow[:, 0:H], in_=tpa[:])
                    tpb = s0_pt.tile([P, P], bf16, space="PSUM", tag="tpb")
                    nc.tensor.transpose(out=tpb[:], in_=p2T[:, tt * P:(tt + 1) * P],
                                        identity=ident_bf[:])
                    nc.scalar.copy(out=xrow[:, H:2 * H], in_=tpb[:])
                    if gt == ZROW_M % MSL // P and True:
                        # patch local row (ZROW_M%MSL)%P of this tile (slot 0 of tile 72)
                        nc.vector.scalar_tensor_tensor(
                            out=xrow[0:1, :], in0=xrow[0:1, :], scalar=zmask_t[0:1, 0:1],
                            in1=zfix_t[0:1, :], op0=mybir.AluOpType.mult,
                            op1=mybir.AluOpType.add)
                    nc.sync.dma_start(out=xcat_slice[gt * P:(gt + 1) * P, :], in_=xrow[:])

        nc.gpsimd.collective_compute(
            "AllGather", mybir.AluOpType.bypass, replica_groups=rg,
            ins=[xcat_slice[:].opt()], outs=[xcat_full[:].opt()])

        # ---------- generic aggregation pass ----------
        def agg_pass(name, order, loc_dram, gidx_dram, n_stream, table_full, tstart,
                     tend, width, n_rng, n_tiles, epilogue, psum_tags):
            """order: list of (coh, r, t, nch). width: rhs free size (256 or 128)."""
            groupsAll = _gather_groups(order, key_fn=lambda seg: (seg[0], seg[1]))
            # chunk -> (group first chunk, tile index of buffer)
            with tc.tile_pool(name=f"{name}_sb", bufs=1) as sbp, \
                 tc.tile_pool(name=f"{name}_s4", bufs=4) as s4p, \
                 tc.tile_pool(name=f"{name}_g", bufs=3) as gp_, \
                 tc.tile_pool(name=f"{name}_gi", bufs=3) as gip, \
                 tc.tile_pool(name=f"{name}_acc", bufs=1, space="PSUM") as accp, \
                 tc.tile_pool(name=f"{name}_eps", bufs=1, space="PSUM") as epsp, \
                 tc.tile_pool(name=f"{name}_est", bufs=3) as estp:
                nsp = (n_stream + 3) // 4 * 4
                loc_t = sbp.tile([P, nsp], bf16)
                nc.sync.dma_start(out=loc_t[:], in_=loc_dram[:])

                # gather buffers, keyed by chunk
                gbufs = {}
                for _i in range(3):
                    zb = gp_.tile([P, GROUP * width], bf16, tag="gb", name="gb")
                    nc.vector.memset(zb[:], 0.0)

                def ensure_gathers(key):
                    for (k_, c0, gn) in groupsAll:
                        if k_ != key:
                            continue
                        r = k_[1]
                        gb = gp_.tile([P, GROUP * width], bf16, tag="gb", name="gb")
                        gi = gip.tile([P, GROUP * 8], i16, tag="gi", name="gi")
                        col0 = c0 * 8
                        nc.sync.dma_start(out=gi[:, :gn * 8],
                                          in_=gidx_dram[:, col0: col0 + gn * 8])
                        nc.gpsimd.dma_gather(
                            out_ap=gb[:, :gn * width].rearrange("p (c n) -> p c n", c=gn),
                            in_ap=table_full[tstart[r]:tend[r], :],
                            idxs_ap=gi[:, :gn * 8],
                            num_idxs=gn * P, num_idxs_reg=gn * P, elem_size=width)
                        for k in range(gn):
                            gbufs[c0 + k] = (gb, k)

                # s4 one-hot cache, keyed by 4-chunk group id
                s4cache = {}

                def get_s4(chunk):
                    g4 = chunk // 4
                    if g4 not in s4cache:
                        s4 = s4p.tile([P, 4 * P], bf16, tag="s4")
                        cc = g4 * 4
                        nc.vector.tensor_tensor(
                            out=s4[:].rearrange("p (k n) -> p k n", k=4),
                            in0=iota_t[:].rearrange("p (k n) -> p k n", k=4),
                            in1=loc_t[:, cc: cc + 4][:, :, None].to_broadcast([P, 4, P]),
                            op=mybir.AluOpType.is_equal)
                        s4cache[g4] = s4
                    return s4cache[g4]

                # per-cohort processing
                n_coh = (n_tiles + COH - 1) // COH
                # segment bookkeeping: positions in stream
                seg_pos = {}
                pos = 0
                for (co, r, t, nch) in order:
                    seg_pos[(r, t)] = (pos, nch)
                    pos += nch

                issued_spans = set()

                pos = 0
                oi = 0
                for co in range(n_coh):
                    tiles = list(range(co * COH, min((co + 1) * COH, n_tiles)))
                    accs = {t: accp.tile([P, width], f32, space="PSUM", name=f"acc{t}",
                                         tag=f"acc{t % COH}") for t in tiles}
                    # last (r, chunk) per tile for stop flag
                    last_chunk = {}
                    first_chunk = {}
                    for r in range(n_rng):
                        for t in tiles:
                            p0, nch = seg_pos[(r, t)]
                            if nch == 0:
                                continue
                            if t not in first_chunk:
                                first_chunk[t] = p0
                            last_chunk[t] = p0 + nch - 1
                    for r in range(n_rng):
                        if (co, r) not in issued_spans:
                            ensure_gathers((co, r))
                            issued_spans.add((co, r))
                        for t in tiles:
                            p0, nch = seg_pos[(r, t)]
                            for i in range(nch):
                                ch = p0 + i
                                gb, slot = gbufs[ch]
                                s4 = get_s4(ch)
                                nc.tensor.matmul(
                                    out=accs[t][:],
                                    lhsT=s4[:, (ch % 4) * P:(ch % 4 + 1) * P],
                                    rhs=gb[:, slot * width:(slot + 1) * width],
                                    start=(ch == first_chunk[t]),
                                    stop=(ch == last_chunk[t]))
                    for t in tiles:
                        epilogue(t, accs[t], epsp, estp)
                        # release gather bufs for this cohort implicitly by pool reuse
                    # drop references so pools can recycle
                    for k in [k for k, v in list(gbufs.items())]:
                        pass

            return

        # ---------- pass B ----------
        def epilogue_B(t, acc, epsp, estp):
            rc = ru_t[:, t:t + 1]
            uh = estp.tile([P, H], bf16, tag="uh")
            nc.scalar.activation(out=uh[:], in_=acc[:, 0:H], func=AF.Relu, scale=rc)
            tp = epsp.tile([P, P], bf16, space="PSUM", tag="tp")
            nc.tensor.transpose(out=tp[:], in_=uh[:], identity=ident_bf[:])
            uht = estp.tile([P, P], bf16, tag="uht")
            nc.vector.tensor_copy(out=uht[:], in_=tp[:])
            psh = epsp.tile([P, H], f32, space="PSUM", tag="ps2")
            nc.tensor.matmul(out=psh[:], lhsT=uht[:], rhs=wtile["wl2um"][:],
                             start=True, stop=False)
            nc.tensor.matmul(out=psh[:], lhsT=ones1_bf[:], rhs=b3_t[:],
                             start=False, stop=True)
            uhsb = estp.tile([P, H], bf16, tag="uhsb")
            nc.vector.tensor_copy(out=uhsb[:], in_=psh[:])
            nc.sync.dma_start(out=userh_sl[t * P:(t + 1) * P, :], in_=uhsb[:])
            psr = epsp.tile([P, H], f32, space="PSUM", tag="ps2")
            nc.tensor.matmul(out=psr[:], lhsT=uht[:], rhs=wtile["wr2mu"][:],
                             start=True, stop=True)
            uo1 = estp.tile([P, H], bf16, tag="uo1")
            nc.scalar.activation(out=uo1[:], in_=acc[:, H:2 * H], func=AF.Copy, scale=rc)
            nc.vector.tensor_tensor(out=uo_sb[:, t * H:(t + 1) * H], in0=uo1[:],
                                    in1=psr[:], op=mybir.AluOpType.add)

        with nc.named_scope("passB"):
            agg_pass("pb", orderB, b_loc, b_gidx, NB, xcat_full, MRS, MRE,
                     2 * H, NRM, UT, epilogue_B, None)

        nc.gpsimd.collective_compute(
            "AllGather", mybir.AluOpType.bypass, replica_groups=rg,
            ins=[userh_sl[:].opt()], outs=[userh_full[:].opt()])

        # ---------- pass C ----------
        def epilogue_C(t, acc, epsp, estp):
            rc = rm_t[:, t:t + 1]
            psr = epsp.tile([P, H], f32, space="PSUM", tag="psr")
            nc.tensor.matmul(out=psr[:], lhsT=mht[:, t * P:(t + 1) * P],
                             rhs=wtile["wr2um"][:], start=True, stop=True)
            mo1 = estp.tile([P, H], bf16, tag="mo1")
            nc.scalar.activation(out=mo1[:], in_=acc[:], func=AF.Copy, scale=rc)
            mo2 = estp.tile([P, H], bf16, tag="mo2")
            nc.vector.tensor_tensor(out=mo2[:], in0=mo1[:], in1=psr[:],
                                    op=mybir.AluOpType.add)
            nc.sync.dma_start(out=mo_sl[t * P:(t + 1) * P, :], in_=mo2[:])

        with nc.named_scope("passC"):
            agg_pass("pc", orderC, c_loc, c_gidx, NC, userh_full,
                     [URS[r] for r in range(NRU)],
                     [URS[r] + URSZ for r in range(NRU)],
                     H, NRU, MT, epilogue_C, None)

        nc.gpsimd.collective_compute(
            "AllGather", mybir.AluOpType.bypass, replica_groups=rg,
            ins=[mo_sl[:].opt()], outs=[mo_full[:].opt()])

        # ---------- pass D ----------
        with nc.named_scope("passD"), \
             tc.tile_pool(name="pd_sb", bufs=1) as pdsb, \
             tc.tile_pool(name="pd_ul", bufs=2) as pdul, \
             tc.tile_pool(name="pd_g", bufs=3) as pdg, \
             tc.tile_pool(name="pd_gi", bufs=3) as pdgi, \
             tc.tile_pool(name="pd_ps", bufs=3, space="PSUM") as pdps, \
             tc.tile_pool(name="pd_st", bufs=4) as pdst:
            outstrip = pdsb.tile([P, ND], f32)

            # gather spans: contiguous (r) ranges
            groupsD = _gather_groups(orderD, key_fn=lambda seg: seg[0])
            gbufsD = {}
            for _i in range(3):
                zb = pdg.tile([P, GROUP * H], bf16, tag="gb", name="gb")
                nc.vector.memset(zb[:], 0.0)

            def ensure_gathers_D(r):
                for (k_, c0, gn) in groupsD:
                    if k_ != r:
                        continue
                    gb = pdg.tile([P, GROUP * H], bf16, tag="gb", name="gb")
                    gi = pdgi.tile([P, GROUP * 8], i16, tag="gi", name="gi")
                    col0 = c0 * 8
                    nc.sync.dma_start(out=gi[:, :gn * 8],
                                      in_=d_mgidx[:, col0: col0 + gn * 8])
                    nc.gpsimd.dma_gather(
                        out_ap=gb[:, :gn * H].rearrange("p (c n) -> p c n", c=gn),
                        in_ap=mo_full[MRS[r]:MRE[r], :],
                        idxs_ap=gi[:, :gn * 8],
                        num_idxs=gn * P, num_idxs_reg=gn * P, elem_size=H)
                    for k in range(gn):
                        gbufsD[c0 + k] = (gb, k)

            # uloc row pieces of 32 chunks
            ULW = 32

            ul_cache = {}

            def get_ul(chunk):
                blk = chunk // ULW
                if blk not in ul_cache:
                    w = min(ULW * P, ND * P - blk * ULW * P)
                    ul = pdul.tile([1, ULW * P], bf16, tag="ul")
                    nc.sync.dma_start(out=ul[0:1, :w],
                                      in_=d_uloc[0:1, blk * ULW * P: blk * ULW * P + w])
                    ul_cache[blk] = ul
                return ul_cache[blk]

            pos = 0
            issuedD = set()
            for (r, t, nch) in orderD:
                if r not in issuedD:
                    ensure_gathers_D(r)
                    issuedD.add(r)
                for i in range(nch):
                    ch = pos + i
                    gbm, slot = gbufsD[ch]
                    ul = get_ul(ch)
                    off = (ch % ULW) * P
                    psb = pdps.tile([P, P], f32, space="PSUM", tag="bc")
                    nc.tensor.matmul(out=psb[:], lhsT=ones1_bf[:],
                                     rhs=ul[0:1, off:off + P], start=True, stop=True)
                    E = pdst.tile([P, P], bf16, tag="E")
                    nc.vector.tensor_tensor(out=E[:], in0=piota_t[:], in1=psb[:],
                                            op=mybir.AluOpType.is_equal)
                    gu = pdps.tile([P, H], f32, space="PSUM", tag="gu")
                    nc.tensor.matmul(out=gu[:], lhsT=E[:],
                                     rhs=uo_sb[:, t * H:(t + 1) * H], start=True, stop=True)
                    scr = pdst.tile([P, H], bf16, tag="scr")
                    nc.vector.scalar_tensor_tensor(
                        out=scr[:], in0=gu[:], scalar=1.0,
                        in1=gbm[:, slot * H:(slot + 1) * H],
                        op0=mybir.AluOpType.mult, op1=mybir.AluOpType.mult,
                        accum_out=outstrip[:, ch:ch + 1])
                pos += nch
            nc.sync.dma_start(out=out[:], in_=outstrip[:])

    nc.compile()
    return nc


# ---------------- entry point ----------------

_CACHE = {}
TRACE = False
LAST_EXEC_NS = None
LAST_RESULTS = None


def kernel(movie_feats, user_init, edge_src, edge_dst, lbl_user, lbl_movie, n_users,
           Wm, bm,
           Wl1_um, bl1_um, Wr1_um, Wl1_mu, bl1_mu, Wr1_mu,
           Wl2_um, bl2_um, Wr2_um, Wl2_mu, bl2_mu, Wr2_mu):
    movie_feats = np.asarray(movie_feats, dtype=np.float32)
    u0 = np.asarray(user_init, np.float32)
    S, Dreal = preprocess(edge_src, edge_dst, lbl_user, lbl_movie)

    key = (S["NB"], S["NC"], S["ND"],
           S["chunksB"].tobytes(), S["chunksC"].tobytes(), S["chunksD"].tobytes())
    if key in _CACHE:
        nc = _CACHE[key]
    else:
        nc = build_program(S)
        _CACHE[key] = nc

    featsT = np.zeros((FD, NMP), npbf16)
    featsT[:, :NM] = movie_feats.T.astype(npbf16)

    # folded biases (host): B1 = bl1_mu + u0 @ Wr1_mu ; B2 = bl2_mu ; B3 = bl2_um
    B1 = (np.asarray(bl1_mu, np.float64) +
          u0.astype(np.float64) @ np.asarray(Wr1_mu, np.float64)).astype(np.float32)
    B2 = np.asarray(bl2_mu, np.float32)
    B3 = np.asarray(bl2_um, np.float32)

    iota_rep = np.tile(np.arange(P, dtype=np.float32)[None, :], (P, 4)).astype(npbf16)
    piota = np.tile(np.arange(P, dtype=np.float32)[:, None], (1, P))

    weights = {
        "wm4": np.asarray(Wm, np.float32).astype(npbf16),
        "wl1mu": np.asarray(Wl1_mu, np.float32).astype(npbf16),
        "wr1um": np.asarray(Wr1_um, np.float32).astype(npbf16),
        "wl2mu": np.asarray(Wl2_mu, np.float32).astype(npbf16),
        "wl2um": np.asarray(Wl2_um, np.float32).astype(npbf16),
        "wr2mu": np.asarray(Wr2_mu, np.float32).astype(npbf16),
        "wr2um": np.asarray(Wr2_um, np.float32).astype(npbf16),
        "bm_col": np.asarray(bm, np.float32),
        "b1_col": B1,
        "bl1um_col": np.asarray(bl1_um, np.float32),
        "b2_col": B2,
        "b3_row": B3.astype(npbf16)[None, :],
        "iota": iota_rep,
        "piota": piota.astype(np.float32),
    }

    zfix_row = np.concatenate([B1, B2]).astype(npbf16)[None, :]

    in_maps = []
    for c in range(W):
        m = {"featsT": np.ascontiguousarray(featsT[:, c * MSL:(c + 1) * MSL])}
        m.update(weights)
        m.update({
            "recip_u": S["recip_u"][c], "recip_m": S["recip_m"][c],
            "zmask": np.array([[0.0 if c == ZROW_M // MSL else 1.0]], np.float32),
            "zfix": zfix_row if c == ZROW_M // MSL else np.zeros((1, 2 * H), npbf16),
            "b_loc": S["b_loc"][c], "b_gidx": S["b_gidx"][c],
            "c_loc": S["c_loc"][c], "c_gidx": S["c_gidx"][c],
            "d_uloc": S["d_uloc"][c], "d_mgidx": S["d_mgidx"][c],
        })
        in_maps.append(m)

    global LAST_EXEC_NS, LAST_RESULTS
    res = run_bass_kernel_spmd(nc, in_maps, core_ids=list(range(W)), trace=TRACE)
    LAST_EXEC_NS = res.exec_time_ns
    LAST_RESULTS = res

    EL = len(np.asarray(lbl_user))
    out_full = np.zeros(EL, np.float32)
    for c in range(W):
        vals = res.results[c]["out"].T.reshape(-1)
        real = Dreal[c]
        mask = real >= 0
        out_full[real[mask]] = vals[mask]
    return out_full
